# revision 41
# baseline (speedup 1.0000x reference)
"""Trainium2 Bass kernel for nn_FAFMoudle (dense_cnn).

Data-parallel across 8 NeuronCores: 32 images per core plus a 3-image halo
on each side for the SSIM uniform filter (which smooths across the batch
axis).  The halo is materialized on the host by symmetrically padding the
global batch, so every core runs an identical program on its own shard.

Device-side plan (per core, all 1x1 convs folded on host into single
matmuls, channel-major layout [C, b*81]):
  pass A: fuse_3/fuse_4 (2ch maps) over the 38 ext images -> SSIM via
          small filter-matrix matmuls (hw-filter 81x81, batch-filter 38x32)
          with PE transposes between; fuse2_2 / cc1(ssim) / xweight
          (fc1+gelu+fc2+leakyrelu) -> linearized per-pixel scalar rows.
  pass B: per 6-image tile: fuse_1/fuse_2 (bf16 matmuls), cosine sims via
          pointwise products + ones-vector PE reductions, fuse2_1/fuse3_1
          chain, xout written into a zero-padded per-image buffer, then the
          3x3 conv as 9*6 accumulating matmuls per output chunk, fused
          BN+leaky-relu on evacuation.
"""

import os
import sys

for _p in (
    "/opt/trn_rl_repo",
    "/root/.axon_site",
    "/root/.axon_site/_ro/trn_rl_repo",
    "/root/.axon_site/_ro/pypackages",
):
    if os.path.isdir(_p) and _p not in sys.path:
        sys.path.insert(0, _p)

import math

import ml_dtypes
import numpy as np

import concourse.bass as bass
import concourse.tile as tile
from concourse import mybir
from concourse.bass_utils import run_bass_kernel_spmd
from concourse.masks import make_identity

dt = mybir.dt
AF = mybir.ActivationFunctionType
ALU = mybir.AluOpType

# ----------------------------------------------------------------------------
# shapes
B, C, L, O, HH, WW = 256, 768, 64, 768, 9, 9
C2, C3 = 2 * C // 3, C // 3
M_CORES = 8
BL = B // M_CORES          # 32 images per core
HALO = 3
BE = BL + 2 * HALO         # 38 ext images
PX = HH * WW               # 81
NV = BL * PX               # 2592 valid pixels
NE = BE * PX               # 3078 ext pixels
KC = C // 128              # 6 contraction chunks
MO = O // 128              # 6 output chunks
G = 6                      # images per pass-B tile
TW = G * PX                # 486
N_TILES = (BL + G - 1) // G
WIN = 7
COV = (WIN ** 3) / (WIN ** 3 - 1.0)
C1S, C2S = 0.01 ** 2, 0.03 ** 2
SQRT_C = math.sqrt(C)
# padded per-image layout for the 3x3 conv input: 11 rows x 12 cols,
# interior at rows 1..9, cols 2..10 (keeps every 9-wide run 4B aligned)
IMR, IMC = 11, 12
IMS = IMR * IMC            # 132

bf16 = dt.bfloat16
f32 = dt.float32
f16 = dt.float16

NPT = 25                   # winograd F(3,3) points per tile (5x5)
NTL = 9                    # 3x3 output tiles per 9x9 image
NPTL = NPT * NTL           # 225 (pt, tile) pairs


def _wino_transforms():
    """F(3,3) 2D Winograd with points {0,1,-1,2} + inf, zero-padding folded
    into the input transform.  Row order: ptile = pt*9 + tile."""
    pts = [0.0, 1.0, -1.0, 2.0]
    V = np.zeros((5, 5))
    V3 = np.zeros((5, 3))
    for i, p in enumerate(pts):
        V[i] = [p ** j for j in range(5)]
        V3[i] = [p ** j for j in range(3)]
    V[4] = [0, 0, 0, 0, 1]
    V3[4] = [0, 0, 1]
    AT = V3.T                      # 3x5
    Gm = V3                        # 5x3 (kernel transform)
    BT = np.linalg.inv(V).T        # 5x5 (input transform)
    Tin = np.zeros((NPTL, 81))
    Tinv = np.zeros((81, NPTL))
    for ti in range(3):
        for tj in range(3):
            tl = ti * 3 + tj
            for pi in range(5):
                for pj in range(5):
                    row = (pi * 5 + pj) * NTL + tl
                    for a in range(5):
                        for b in range(5):
                            r, c = 3 * ti + a - 1, 3 * tj + b - 1
                            if 0 <= r < 9 and 0 <= c < 9:
                                Tin[row, r * 9 + c] += BT[pi, a] * BT[pj, b]
            for oi in range(3):
                for oj in range(3):
                    orow = (3 * ti + oi) * 9 + (3 * tj + oj)
                    for pi in range(5):
                        for pj in range(5):
                            Tinv[orow, (pi * 5 + pj) * NTL + tl] = \
                                AT[oi, pi] * AT[oj, pj]
    return Gm, Tin, Tinv

# BV (bias/const matrix) column map
BV_BH1 = 0          # 6 cols
BV_BF2 = 6          # 6 cols
BV_B4 = 12          # 1 col (rows 0:2, f3 bias)
BV_BFC1 = 13        # 3 cols
BV_BFC2 = 16        # 1 col (rows 0:81)
BV_BNS = 17         # 6 cols
BV_BNB = 23         # 6 cols
BV_W00 = 29
BV_W01 = 30
BV_BCC = 31
BV_BP0 = 32
BV_BP1 = 33
BV_B4Y = 34         # f4 bias (rows 0:2)
BV_NCOLS = 35

_SYNC_WAIT_LIMIT = 1


def _patch_drain_wait_limit():
    """walrus in this container only allows 2 sem waits per TPB_CTRL
    instruction; split the tile-exit drain's waits across extra nops."""
    import bass_rust
    from concourse.tile import ScopedClock, TileContext

    if getattr(TileContext, "_drain_waits_patched", False):
        return

    def _drain_and_barrier(self, tick_clock, wait_clock):
        drain_inst = self.nc.sync.drain()
        wait_clock.add_sem_waits(
            drain_inst.ins, ScopedClock({None: tick_clock.global_clock})
        )
        si = drain_inst.ins.sync_info
        waits = list(si.on_wait)
        if len(waits) > _SYNC_WAIT_LIMIT:
            drain_inst.ins.sync_info = bass_rust.SyncInfo(
                on_wait=waits[:_SYNC_WAIT_LIMIT], on_update=list(si.on_update)
            )
            for i in range(_SYNC_WAIT_LIMIT, len(waits), _SYNC_WAIT_LIMIT):
                n = self.nc.sync.nop()
                n.ins.sync_info = bass_rust.SyncInfo(
                    on_wait=waits[i : i + _SYNC_WAIT_LIMIT], on_update=[]
                )
        self.nc.all_engine_barrier()
        popped = self.nc._tile_sem_poison_stack.pop()
        assert popped is self._sem_poison
        self.nc.clear_and_free_semaphores(list(self.sems.allocated().values()))
        self.nc.all_engine_barrier()

    TileContext._drain_and_barrier = _drain_and_barrier
    TileContext._drain_waits_patched = True


def _emit(ctx, nc, tc, io):
    v = nc.vector
    sc = nc.scalar
    te = nc.tensor

    cp = ctx.enter_context(tc.tile_pool(name="const", bufs=1))
    # "big" tag: pass-A persistent tiles share six 14.4KB slots with the six
    # per-kc winograd point-value tiles (val_ck) that only start filling in
    # pass B, after every pass-A tile is dead
    ovl = ctx.enter_context(tc.tile_pool(name="ovl", bufs=6))
    xt_pool = ctx.enter_context(tc.tile_pool(name="xt", bufs=2))
    f_pool = ctx.enter_context(tc.tile_pool(name="fs", bufs=2))
    prod_pool = ctx.enter_context(tc.tile_pool(name="prod", bufs=4))
    bcs_pool = ctx.enter_context(tc.tile_pool(name="bcs", bufs=2))
    sc_pool = ctx.enter_context(tc.tile_pool(name="sct", bufs=3))
    out_pool = ctx.enter_context(tc.tile_pool(name="outp", bufs=2))
    wA_pool = ctx.enter_context(tc.tile_pool(name="wA", bufs=1))
    ht_pool = ctx.enter_context(tc.tile_pool(name="ht", bufs=2))
    wg_pool = ctx.enter_context(tc.tile_pool(name="wgp", bufs=2))
    vo_pool = ctx.enter_context(tc.tile_pool(name="vo", bufs=1))
    vp_pool = ctx.enter_context(tc.tile_pool(name="vp", bufs=1))

    ps_a = ctx.enter_context(tc.tile_pool(name="psA", bufs=3, space="PSUM"))
    ps_red = ctx.enter_context(tc.tile_pool(name="psRed", bufs=1, space="PSUM"))

    # ---- constants / weights into SBUF --------------------------------
    def ld(name, shape, dtype, ap):
        t = cp.tile(shape, dtype, name=name)
        nc.sync.dma_start(out=t[:], in_=ap)
        return t

    A3X = ld("A3X", [128, KC, 2], bf16,
             io["a3x"].ap().rearrange("(kc p) m -> p kc m", p=128))
    SY4 = ld("SY4", [L, 4], bf16, io["sy4"].ap())
    BV = ld("BV", [128, BV_NCOLS], f32, io["bv"].ap())
    ye_ap = io["ye"].ap()
    xe_re0 = io["xe"].ap().rearrange("(kc p) n -> p kc n", p=128)

    # first pass-A chunk DMAs go out before the bulky consts so the PE can
    # start as soon as possible
    preA = {}

    def loadA(c0, w):
        xa = xt_pool.tile([128, KC, TW], bf16, tag="xt", name=f"xa{c0}")
        nc.sync.dma_start(out=xa[:, :, :w], in_=xe_re0[:, :, c0 : c0 + w])
        ya = xt_pool.tile([L, TW], bf16, tag="yt", name=f"ya{c0}")
        nc.sync.dma_start(out=ya[:, :w], in_=ye_ap[:, c0 : c0 + w])
        return xa, ya

    preA[0] = loadA(0, min(TW, NE))

    WFC1 = ld("WFC1", [81, 324], bf16, io["wfc1"].ap())
    WFC2 = ld("WFC2", [128, 3, 81], bf16,
              io["wfc2"].ap().rearrange("(kc p) m -> p kc m", p=128))
    SHW = ld("SHW", [81, 81], f32, io["shw"].ap())
    SB = ld("SB", [BE, BL], f32, io["sb"].ap())
    TINR = ld("TINR", [81, NPTL], f16, io["tinr"].ap())
    TINVA = ld("TINVA", [128, 81], f16, io["tinva"].ap())
    TINVB = ld("TINVB", [98, 81], f16, io["tinvb"].ap())

    IDF = cp.tile([128, 128], f32, name="IDF")
    make_identity(nc, IDF[:])
    IDFB = cp.tile([128, 128], bf16, name="IDFB")
    make_identity(nc, IDFB[:])
    IDFH = cp.tile([128, 128], f16, name="IDFH")
    make_identity(nc, IDFH[:])
    ONESC = cp.tile([128, 1], bf16, name="ONESC")
    nc.gpsimd.memset(ONESC[:], 1.0)
    ONESR = cp.tile([1, 128], bf16, name="ONESR")
    nc.gpsimd.memset(ONESR[:], 1.0)
    EPSR = cp.tile([1, 1], f32, name="EPSR")
    nc.gpsimd.memset(EPSR[:], 1e-16)

    xe_re = io["xe"].ap().rearrange("(kc p) n -> p kc n", p=128)

    st = {}

    def tdims(g):
        gi = min(G, BL - g * G)
        return gi, gi * PX, g * TW, HALO * PX + g * TW

    def stage_load(g):
        gi, w, c0, ce = tdims(g)
        xt = xt_pool.tile([128, KC, TW], bf16, tag="xt", name=f"xt{g}")
        nc.sync.dma_start(out=xt[:, :, :w], in_=xe_re[:, :, ce : ce + w])
        yt = xt_pool.tile([L, TW], bf16, tag="yt", name=f"yt{g}")
        nc.sync.dma_start(out=yt[:, :w], in_=ye_ap[:, ce : ce + w])
        st[g] = {"xt": xt, "yt": yt}

    def stage_f1(g, ms):
        gi, w, c0, ce = tdims(g)
        s = st[g]
        if "F1S" not in s:
            s["F1S"] = f_pool.tile([128, KC, TW], bf16, tag="f1s",
                                   name=f"f1s{g}")
        F1S = s["F1S"]
        for m in ms:
            p1 = ps_a.tile([128, TW], f32, tag="pa", name=f"p1_{g}_{m}")
            for k in range(KC):
                te.matmul(p1[:, :w], WH1[:, k, m * 128 : (m + 1) * 128],
                          s["xt"][:, k, :w], start=(k == 0),
                          stop=(k == KC - 1))
            sc.activation(F1S[:, m, :w], p1[:, :w], AF.Identity,
                          bias=BV[:, BV_BH1 + m : BV_BH1 + m + 1])

    def stage_f2(g, ms):
        gi, w, c0, ce = tdims(g)
        s = st[g]
        if "F2S" not in s:
            s["F2S"] = f_pool.tile([128, KC, TW], bf16, tag="f2s", bufs=2,
                                   name=f"f2s{g}")
        F2S = s["F2S"]
        for m in ms:
            p2 = ps_a.tile([128, TW], f32, tag="pa", name=f"p2_{g}_{m}")
            te.matmul(p2[:, :w], WF2Y[:, m * 128 : (m + 1) * 128],
                      s["yt"][:, :w], start=True, stop=False)
            for k in range(KC):
                te.matmul(p2[:, :w], WF2X[:, k, m * 128 : (m + 1) * 128],
                          s["xt"][:, k, :w], start=False, stop=(k == KC - 1))
            sc.activation(F2S[:, m, :w], p2[:, :w], AF.Identity,
                          bias=BV[:, BV_BF2 + m : BV_BF2 + m + 1])

    def stage_fold_red(g, which):
        # 6-fold the channel-chunk terms on DVE, then one M=1 matmul into
        # a packed psum row (rows 32-aligned so groups stay independent)
        gi, w, c0, ce = tdims(g)
        s = st[g]
        F1S, F2S = s["F1S"], s["F2S"]
        if "rr" not in s:
            s["rr"] = ps_red.tile([1, 5 * 512], f32, tag="red", name=f"rr_{g}")
        spec = {
            "r1": (0, F1S, F2S),
            "r2": (1, F1S, F1S),
            "r3": (2, F2S, F2S),
            "r6": (3, F1S, None),
            "r7": (4, F2S, None),
        }
        slot, a, b = spec[which]
        rt = s["rr"]
        acc = prod_pool.tile([128, TW], bf16, tag="pp", name=f"ac{which}{g}")
        if b is None:
            v.tensor_add(acc[:, :w], a[:, 0, :w], a[:, 1, :w])
            for m in range(2, MO):
                v.tensor_add(acc[:, :w], acc[:, :w], a[:, m, :w])
        else:
            v.tensor_mul(acc[:, :w], a[:, 0, :w], b[:, 0, :w])
            for m in range(1, MO):
                tmp = prod_pool.tile([128, TW], bf16, tag="pp",
                                     name=f"tp{which}{g}_{m}")
                v.tensor_mul(tmp[:, :w], a[:, m, :w], b[:, m, :w])
                v.tensor_add(acc[:, :w], acc[:, :w], tmp[:, :w])
        te.matmul(rt[0:1, 512 * slot : 512 * slot + w], ONESC[:],
                  acc[:, :w], start=True, stop=True)

    def stage_cor1_q(g):
        # issued right after the r2/r3 folds: the 1/sqrt(r2*r3) chain runs
        # on scalar while the r1 fold is still going on vector
        gi, w, c0, ce = tdims(g)
        s = st[g]
        rr = s["rr"]
        r2 = rr[0:1, 512 : 512 + TW]
        r3 = rr[0:1, 1024 : 1024 + TW]
        q1 = sc_pool.tile([1, TW], f32, tag="scf", bufs=4, name=f"q1_{g}")
        q3 = sc_pool.tile([1, TW], f32, tag="scf", bufs=4, name=f"q3_{g}")
        qs = sc_pool.tile([1, TW], f32, tag="scf", bufs=4, name=f"qs_{g}")
        sc.activation(q3[:, :w], r3[:, :w], AF.Copy)
        v.tensor_mul(qs[:, :w], r2[:, :w], q3[:, :w])
        # 1/sqrt(x) as exp(-0.5*ln(x)) -- keeps the whole chain on the scalar
        # engine instead of DVE's ~3.8us iterative reciprocal
        sc.activation(qs[:, :w], qs[:, :w], AF.Ln, bias=EPSR[0:1, 0:1])
        sc.activation(q1[:, :w], qs[:, :w], AF.Exp, scale=-0.5)
        s["q1"] = q1

    def stage_cor1_bb(g):
        gi, w, c0, ce = tdims(g)
        s = st[g]
        rr = s["rr"]
        r1 = rr[0:1, 0:TW]
        q1 = s["q1"]
        beta = sc_pool.tile([1, TW], bf16, tag="scb", name=f"beta{g}")
        q2 = sc_pool.tile([1, TW], f32, tag="scf", bufs=4, name=f"q2_{g}")
        v.scalar_tensor_tensor(q2[:, :w], r1[:, :w], -0.5, q1[:, :w],
                               ALU.mult, ALU.mult)
        v.tensor_scalar_add(beta[:, :w], q2[:, :w], 0.5)
        s["beta"] = beta
        bb = ps_a.tile([128, TW], f32, tag="pa", name=f"bb{g}")
        te.matmul(bb[:, :w], ONESR[:], beta[:, :w], start=True, stop=True)
        bbs = bcs_pool.tile([128, TW], bf16, tag="bcs", name=f"bbs{g}")
        sc.activation(bbs[:, :w], bb[:, :w], AF.Copy)
        s["bbs"] = bbs

    def stage_algebra(g):
        gi, w, c0, ce = tdims(g)
        s = st[g]
        rr = s["rr"]
        r1 = rr[0:1, 0:TW]
        r2 = rr[0:1, 512 : 512 + TW]
        r3 = rr[0:1, 1024 : 1024 + TW]
        r6 = rr[0:1, 1536 : 1536 + TW]
        r7 = rr[0:1, 2048 : 2048 + TW]
        beta = s["beta"]
        # r4 = r6 + beta*r7   (fuse2_1 channel-sum, no extra reduction)
        r4s = sc_pool.tile([1, TW], f32, tag="scf", bufs=4, name=f"r4s_{g}")
        v.tensor_mul(r4s[:, :w], beta[:, :w], r7[:, :w])
        v.tensor_add(r4s[:, :w], r4s[:, :w], r6[:, :w])
        s["r4s"] = r4s
        # r5 = r2 + 2*beta*r1 + beta^2*r3
        t1 = sc_pool.tile([1, TW], f32, tag="scf", bufs=4, name=f"t1_{g}")
        t2 = sc_pool.tile([1, TW], f32, tag="scf", bufs=4, name=f"t2_{g}")
        v.tensor_mul(t1[:, :w], beta[:, :w], r1[:, :w])
        v.tensor_mul(t2[:, :w], beta[:, :w], r3[:, :w])
        v.tensor_mul(t2[:, :w], beta[:, :w], t2[:, :w])
        v.scalar_tensor_tensor(t1[:, :w], t1[:, :w], 2.0, t2[:, :w],
                               ALU.mult, ALU.add)
        v.tensor_add(t1[:, :w], t1[:, :w], r2[:, :w])
        s["r5s"] = t1

    def stage_fuse21(g):
        gi, w, c0, ce = tdims(g)
        s = st[g]
        F1S, F2S, bbs = s["F1S"], s["F2S"], s["bbs"]
        for m in range(MO):
            td = prod_pool.tile([128, TW], bf16, tag="pp", name=f"td{g}_{m}")
            v.tensor_mul(td[:, :w], bbs[:, :w], F2S[:, m, :w])
            # fuse2_1 overwrites F1S in place
            v.tensor_add(F1S[:, m, :w], td[:, :w], F1S[:, m, :w])

    def stage_lrows(g):
        gi, w, c0, ce = tdims(g)
        s = st[g]
        lr = sc_pool.tile([1, 3, TW], bf16, tag="lrow", bufs=2,
                          name=f"lr{g}")
        for nm_, idx in (("f22l", 0), ("sccl", 1), ("xwl", 2)):
            nc.sync.dma_start(
                out=lr[0:1, idx, :w],
                in_=lin_scr[idx].ap().rearrange(
                    "(one b) q -> one (b q)", one=1)[:, c0 : c0 + w])
        s["f22l"] = lr[0:1, 0, :]
        s["sccl"] = lr[0:1, 1, :]
        s["xwl"] = lr[0:1, 2, :]

    def stage_cor2(g):
        gi, w, c0, ce = tdims(g)
        s = st[g]
        r4s, r5s = s["r4s"], s["r5s"]
        f22l = s["f22l"]
        nmr = sc_pool.tile([1, TW], f32, tag="scf", bufs=4, name=f"nm{g}")
        v.tensor_mul(nmr[:, :w], f22l[:, :w], r4s[:, :w])
        # 1/(sqrt(r5)*|f22l|*sqrt(C)) = exp(-0.5*ln(r5*f22l^2*C))
        s5 = sc_pool.tile([1, TW], f32, tag="scf", bufs=4, name=f"s5_{g}")
        af_ = sc_pool.tile([1, TW], f32, tag="scf", bufs=4, name=f"af{g}")
        v.tensor_mul(af_[:, :w], f22l[:, :w], f22l[:, :w])
        v.tensor_mul(s5[:, :w], r5s[:, :w], af_[:, :w])
        sc.activation(s5[:, :w], s5[:, :w], AF.Ln, scale=float(C), bias=EPSR[0:1, 0:1])
        s5i = sc_pool.tile([1, TW], f32, tag="scf", bufs=4, name=f"s5i_{g}")
        sc.activation(s5i[:, :w], s5[:, :w], AF.Exp, scale=-0.5)
        v.tensor_mul(nmr[:, :w], nmr[:, :w], s5i[:, :w])    # cor2
        v.tensor_sub(nmr[:, :w], nmr[:, :w], s["sccl"][:, :w])
        v.tensor_scalar(nmr[:, :w], nmr[:, :w], -0.5, 0.5, ALU.mult, ALU.add)
        delta = sc_pool.tile([1, TW], bf16, tag="scb", name=f"dl{g}")
        v.tensor_mul(delta[:, :w], nmr[:, :w], f22l[:, :w])
        s["delta"] = delta
        xw1 = sc_pool.tile([1, TW], bf16, tag="scb", name=f"xw1_{g}")
        v.tensor_scalar_add(xw1[:, :w], s["xwl"][:, :w], 1.0)
        s["xw1"] = xw1

    def stage_cor2_bcast(g):
        gi, w, c0, ce = tdims(g)
        s = st[g]
        bd = ps_a.tile([128, TW], f32, tag="pa", name=f"bd{g}")
        te.matmul(bd[:, :w], ONESR[:], s["delta"][:, :w], start=True,
                  stop=True)
        dbs = bcs_pool.tile([128, TW], bf16, tag="bcs", name=f"dbs{g}")
        sc.activation(dbs[:, :w], bd[:, :w], AF.Copy)
        s["dbs"] = dbs
        bw = ps_a.tile([128, TW], f32, tag="pa", name=f"bw{g}")
        te.matmul(bw[:, :w], ONESR[:], s["xw1"][:, :w], start=True, stop=True)
        wbs = bcs_pool.tile([128, TW], bf16, tag="bcs", name=f"wbs{g}")
        sc.activation(wbs[:, :w], bw[:, :w], AF.Copy)
        s["wbs"] = wbs

    def stage_h(g):
        # h = (fuse2_1 + delta_bcast) * (1 + xweight)_bcast, in place in F1S
        gi, w, c0, ce = tdims(g)
        s = st[g]
        F1S, dbs, wbs = s["F1S"], s["dbs"], s["wbs"]
        for m in range(MO):
            v.tensor_add(F1S[:, m, :w], F1S[:, m, :w], dbs[:, :w])
            v.tensor_mul(F1S[:, m, :w], F1S[:, m, :w], wbs[:, :w])

    def stage_w1w2(g, i):
        # one image: transpose h to pixel-major, then the fused winograd
        # input transform with the image data as lhsT -> val_ck channel-major
        gi, w, c0, ce = tdims(g)
        if i >= gi:
            return
        s = st[g]
        F1S = s["F1S"]
        im = g * G + i
        HT = ht_pool.tile([81, KC, 128], f16, tag="ht", name=f"ht{g}_{i}")
        for half in range(2):
            tp = ps_a.tile([81, 3 * 128], bf16, tag="pa",
                           name=f"w1p{g}_{i}_{half}")
            for k in range(3):
                m = half * 3 + k
                te.transpose(tp[:, k * 128 : (k + 1) * 128],
                             F1S[:, m, i * PX : i * PX + PX], IDFB[:])
            sc.activation(HT[:, half * 3 : half * 3 + 3, :],
                          tp[:].rearrange("p (k c) -> p k c", c=128), AF.Copy)
        for k in range(KC):
            tq = ps_a.tile([128, NPTL], f32, tag="pa", name=f"w2p{g}_{i}_{k}")
            te.matmul(tq[:], HT[:, k, :], TINR[:], start=True, stop=True)
            eng = sc if k % 2 == 0 else v
            if eng is sc:
                sc.activation(val_ck[k][:, :, :, im],
                              tq[:].rearrange("p (pt t) -> p pt t", t=NTL),
                              AF.Copy)
            else:
                v.tensor_copy(val_ck[k][:, :, :, im],
                              tq[:].rearrange("p (pt t) -> p pt t", t=NTL))

    def phase2():
        wg_re = io["wg"].ap()         # [MO, 25, 128, KC, 128]
        bnb_re = io["bnbrep"].ap()    # [MO, BL*128]
        out2_re = io["out"].ap()      # [81, BL, O]
        FW = BL * 128                 # 4096 inverse-transform columns per mo
        CHW = 384                     # 3 images per chunk (psum-bank sized)
        nch = (FW + CHW - 1) // CHW
        for mo in range(MO):
            vo = vo_pool.tile([128, NPT, NTL, BL], f16, tag="vo",
                              name=f"vo{mo}")
            for pt in range(NPT):
                wgt = wg_pool.tile([128, KC, 128], f16, tag="wg",
                                   name=f"wg{mo}_{pt}")
                nc.sync.dma_start(out=wgt[:], in_=wg_re[mo, pt])
                pq = ps_a.tile([128, NTL * BL], f32, tag="pa",
                               name=f"pq{mo}_{pt}")
                for k in range(KC):
                    te.matmul(pq[:], wgt[:, k, :], val_ck[k][:, pt, :, :],
                              start=(k == 0), stop=(k == KC - 1))
                if pt % 2 == 0:
                    sc.activation(
                        vo[:, pt, :, :],
                        pq[:].rearrange("p (t b) -> p t b", b=BL), AF.Copy)
                else:
                    v.tensor_copy(
                        vo[:, pt, :, :],
                        pq[:].rearrange("p (t b) -> p t b", b=BL))
            HB = BL // 2
            for hf in range(2):
                vpA = vp_pool.tile([128, HB, 128], f16, tag="vpa",
                                   name=f"vpa{mo}_{hf}")
                vpB = vp_pool.tile([98, HB, 128], f16, tag="vpb",
                                   name=f"vpb{mo}_{hf}")
                nc.sync.dma_start(
                    out=vpB[97:98, :, :].rearrange("o b c -> o (b c)"),
                    in_=bnb_re[mo : mo + 1,
                               hf * HB * 128 : (hf + 1) * HB * 128])
                for i in range(HB):
                    im = hf * HB + i
                    va = vo[:, :, :, im].rearrange("p pt t -> p (pt t)")
                    t5a = ps_a.tile([128, 128], f16, tag="pa",
                                    name=f"t5a{mo}_{im}")
                    te.transpose(t5a[:], va[:, 0:128], IDFH[:])
                    sc.activation(vpA[:, i, :], t5a[:], AF.Copy)
                    t5b = ps_a.tile([97, 128], f16, tag="pa",
                                    name=f"t5b{mo}_{im}")
                    te.transpose(t5b[:], va[:, 128:NPTL], IDFH[:])
                    v.tensor_copy(vpB[0:97, i, :], t5b[:])
                vaf = vpA[:].rearrange("p b c -> p (b c)")
                vbf = vpB[:].rearrange("p b c -> p (b c)")
                FWH = HB * 128
                nch = (FWH + CHW - 1) // CHW
                for ch in range(nch):
                    f0 = ch * CHW
                    fw = min(CHW, FWH - f0)
                    tv = ps_a.tile([81, CHW], f32, tag="pa",
                                   name=f"ti{mo}_{hf}_{ch}")
                    te.matmul(tv[:, :fw], TINVA[:], vaf[:, f0 : f0 + fw],
                              start=True, stop=False)
                    te.matmul(tv[:, :fw], TINVB[:], vbf[:, f0 : f0 + fw],
                              start=False, stop=True)
                    ob = out_pool.tile([81, CHW], f32, tag="ot",
                                       name=f"ob{mo}_{hf}_{ch}")
                    sc.activation(ob[:, :fw], tv[:, :fw], AF.Lrelu,
                                  alpha=0.01)
                    b0 = hf * HB + 3 * ch
                    nc.sync.dma_start(
                        out=out2_re[:, b0 : b0 + fw // 128,
                                    mo * 128 : (mo + 1) * 128],
                        in_=ob[:, :fw].rearrange("p (b c) -> p b c", c=128))


    # =========================== pass A ================================
    # fuse_3 / fuse_4 over ext pixels, transposed per image into
    # T34 [81, (t, b)] with t in {f3c0, f3c1, f4c0, f4c1}
    T34 = ovl.tile([81, 4, BE], f32, tag="big", name="T34")
    chunksA = [(c0, min(TW, NE - c0)) for c0 in range(0, NE, TW)]
    for c0, w in chunksA:
        nb = w // PX
        b0 = c0 // PX
        if c0 in preA:
            xa, ya = preA.pop(c0)
        else:
            xa, ya = loadA(c0, w)
        f3p = ps_a.tile([2, TW], f32, tag="pa", name=f"f3p{c0}")
        f4p = ps_a.tile([2, TW], f32, tag="pa", name=f"f4p{c0}")
        te.matmul(f4p[:, :w], SY4[:, 2:4], ya[:, :w], start=True, stop=True)
        te.matmul(f3p[:, :w], SY4[:, 0:2], ya[:, :w], start=True, stop=False)
        for k in range(KC):
            te.matmul(f3p[:, :w], A3X[:, k, :], xa[:, k, :w],
                      start=False, stop=(k == KC - 1))
        f3s = xt_pool.tile([2, TW], f32, tag="f3s", bufs=1, name=f"f3s{c0}")
        f4s = xt_pool.tile([2, TW], f32, tag="f4s", bufs=1, name=f"f4s{c0}")
        sc.activation(f3s[:, :w], f3p[:, :w], AF.Identity,
                      bias=BV[0:2, BV_B4 : BV_B4 + 1])
        sc.activation(f4s[:, :w], f4p[:, :w], AF.Identity,
                      bias=BV[0:2, BV_B4Y : BV_B4Y + 1])
        pt = ps_a.tile([81, 4 * G], f32, tag="pa", name=f"pt{c0}")
        for i in range(nb):
            te.transpose(pt[:, 4 * i : 4 * i + 2],
                         f3s[:, i * 81 : (i + 1) * 81], IDF[0:2, 0:2])
            te.transpose(pt[:, 4 * i + 2 : 4 * i + 4],
                         f4s[:, i * 81 : (i + 1) * 81], IDF[0:2, 0:2])
        sc.activation(
            T34[:, :, b0 : b0 + nb].rearrange("p t b -> p b t"),
            pt[:, : 4 * nb].rearrange("p (b t) -> p b t", t=4),
            AF.Copy)

    # fuse weights land after the pass-A x chunks are in flight
    WH1 = ld("WH1", [128, KC, C], bf16,
             io["wh1"].ap().rearrange("(kc p) m -> p kc m", p=128))
    WF2X = ld("WF2X", [128, KC, C], bf16,
              io["wf2x"].ap().rearrange("(kc p) m -> p kc m", p=128))
    WF2Y = ld("WF2Y", [L, C], bf16, io["wf2y"].ap())
    stage_load(0)
    stage_load(1)

    # -- A1: products + hw-filter ------------------------------------
    U_IN = ovl.tile([81, 10, BE], f32, tag="big", name="U_IN")
    v.tensor_copy(U_IN[:, 0:4, :], T34[:, :, :])
    for c in range(2):
        s_ = T34[:, c, :]
        t_ = T34[:, 2 + c, :]
        v.tensor_mul(U_IN[:, 4 + c, :], s_, s_)
        v.tensor_mul(U_IN[:, 6 + c, :], t_, t_)
        v.tensor_mul(U_IN[:, 8 + c, :], s_, t_)
    psU = ps_a.tile([81, 10 * BE], f32, tag="pa", name="psU")
    te.matmul(psU[:], SHW[:], U_IN[:, :, :], start=True, stop=True)
    UF = ovl.tile([81, 10, BE], f32, tag="big", name="UF")
    sc.activation(UF[:, :, :], psU[:].rearrange("p (m b) -> p m b", b=BE),
                  AF.Copy)

    stage_f1(0, [0, 1, 2])

    # -- A2: reverse transposes --------------------------------------
    UT = ovl.tile([BE, 10, 81], f32, tag="big", name="UT")
    for m0 in range(0, 10, 6):
        nm = min(6, 10 - m0)
        pt2 = ps_a.tile([BE, 6 * 81], f32, tag="pa", name=f"pt2{m0}")
        for i in range(nm):
            te.transpose(pt2[:, 81 * i : 81 * (i + 1)],
                         UF[:, m0 + i, :], IDF[0:81, 0:81])
        sc.activation(UT[:, m0 : m0 + nm, :],
                      pt2[:, : 81 * nm].rearrange("p (m q) -> p m q", q=81),
                      AF.Copy)
    TT34 = ovl.tile([BL, 4, 81], f32, tag="big", name="TT34")
    pt3 = ps_a.tile([BL, 4 * 81], f32, tag="pa", name="pt3")
    for i in range(4):
        te.transpose(pt3[:, 81 * i : 81 * (i + 1)],
                     T34[:, i, HALO : HALO + BL], IDF[0:81, 0:81])
    sc.activation(TT34[:, :, :],
                  pt3[:].rearrange("p (m q) -> p m q", q=81), AF.Copy)

    stage_f1(0, [3, 4, 5])

    # -- A3: batch filter --------------------------------------------
    UU = ovl.tile([BL, 10, 81], f32, tag="big", name="UU")
    for m0 in range(0, 10, 5):
        pu = ps_a.tile([BL, 5 * 81], f32, tag="pa", name=f"pu{m0}")
        for i in range(5):
            te.matmul(pu[:, 81 * i : 81 * (i + 1)], SB[:], UT[:, m0 + i, :],
                      start=True, stop=True)
        sc.activation(UU[:, m0 : m0 + 5, :],
                      pu[:].rearrange("p (m q) -> p m q", q=81), AF.Copy)

    stage_f1(1, [0, 1, 2])

    # -- A4: ssim arithmetic -----------------------------------------
    SS = ovl.tile([BL, 2, 81], f32, tag="big", name="SS")
    Z = ovl.tile([BL, 2, 81], f32, tag="big", name="Z")
    for c in range(2):
        ux, uy = UU[:, c, :], UU[:, 2 + c, :]
        uxx, uyy, uxy = UU[:, 4 + c, :], UU[:, 6 + c, :], UU[:, 8 + c, :]
        w1 = wA_pool.tile([BL, 81], f32, tag="wa", bufs=6, name=f"w1c{c}")
        w2 = wA_pool.tile([BL, 81], f32, tag="wa", bufs=6, name=f"w2c{c}")
        w3 = wA_pool.tile([BL, 81], f32, tag="wa", bufs=6, name=f"w3c{c}")
        w4 = wA_pool.tile([BL, 81], f32, tag="wa", bufs=6, name=f"w4c{c}")
        w5 = wA_pool.tile([BL, 81], f32, tag="wa", bufs=6, name=f"w5c{c}")
        v.tensor_mul(w1[:], ux, uy)
        v.tensor_mul(w2[:], ux, ux)
        v.tensor_mul(w3[:], uy, uy)
        v.tensor_add(w4[:], w2[:], w3[:])
        v.tensor_scalar(w2[:], w1[:], 2.0, C1S, ALU.mult, ALU.add)
        v.tensor_sub(w3[:], uxy, w1[:])
        v.tensor_scalar(w1[:], w3[:], 2.0 * COV, C2S, ALU.mult, ALU.add)
        v.tensor_scalar(w3[:], w4[:], 1.0, C1S, ALU.mult, ALU.add)
        v.tensor_add(w5[:], uxx, uyy)
        v.tensor_sub(w5[:], w5[:], w4[:])
        v.tensor_scalar(w5[:], w5[:], COV, C2S, ALU.mult, ALU.add)
        v.tensor_mul(w2[:], w2[:], w1[:])
        v.tensor_mul(w3[:], w3[:], w5[:])
        w6 = wA_pool.tile([BL, 81], f32, tag="wa", bufs=6, name=f"w6c{c}")
        sc.activation(w3[:], w3[:], AF.Ln)
        sc.activation(w6[:], w3[:], AF.Exp, scale=-1.0)
        v.tensor_mul(SS[:, c, :], w2[:], w6[:])
        v.tensor_mul(w1[:], SS[:, c, :], TT34[:, c, :])
        v.tensor_add(Z[:, c, :], w1[:], TT34[:, 2 + c, :])

    F22T = ovl.tile([BL, 81], f32, tag="big", name="F22T")
    SSCC = ovl.tile([BL, 81], f32, tag="big", name="SSCC")
    wz = wA_pool.tile([BL, 81], f32, tag="wa", bufs=6, name="wz")
    v.tensor_scalar(wz[:], Z[:, 1, :], BV[0:BL, BV_W01 : BV_W01 + 1],
                    BV[0:BL, BV_BCC : BV_BCC + 1], ALU.mult, ALU.add)
    v.scalar_tensor_tensor(F22T[:], Z[:, 0, :],
                           BV[0:BL, BV_W00 : BV_W00 + 1], wz[:],
                           ALU.mult, ALU.add)
    wz2 = wA_pool.tile([BL, 81], f32, tag="wa", bufs=6, name="wz2")
    v.tensor_scalar(wz2[:], SS[:, 1, :], BV[0:BL, BV_W01 : BV_W01 + 1],
                    BV[0:BL, BV_BCC : BV_BCC + 1], ALU.mult, ALU.add)
    v.scalar_tensor_tensor(SSCC[:], SS[:, 0, :],
                           BV[0:BL, BV_W00 : BV_W00 + 1], wz2[:],
                           ALU.mult, ALU.add)

    stage_f1(1, [3, 4, 5])

    # -- A5: fc1 (pool conv folded on host) --------------------------
    ptr = ps_a.tile([81, BL], f32, tag="pa", name="ptrF22")
    te.transpose(ptr[:], F22T[:], IDF[0:BL, 0:BL])
    F22TT = ovl.tile([81, BL], bf16, tag="big", name="F22TT")
    sc.activation(F22TT[:], ptr[:], AF.Copy)

    H1S = ovl.tile([128, 3, BL], bf16, tag="big", name="H1S")
    nc.gpsimd.memset(H1S[:], 0.0)
    for mo in range(3):
        osz = min(128, 324 - mo * 128)
        pf = ps_a.tile([128, BL], f32, tag="pa", name=f"pf{mo}")
        te.matmul(pf[0:osz, :], WFC1[:, mo * 128 : mo * 128 + osz],
                  F22TT[:], start=True, stop=True)
        sc.activation(H1S[0:osz, mo, :], pf[0:osz, :], AF.Gelu,
                      bias=BV[0:osz, BV_BFC1 + mo : BV_BFC1 + mo + 1])

    stage_f2(0, [0, 1, 2])

    # -- A6: fc2 + leaky + linearize ---------------------------------
    pxw = ps_a.tile([81, BL], f32, tag="pa", name="pxw")
    for k in range(3):
        te.matmul(pxw[:], WFC2[:, k, :], H1S[:, k, :],
                  start=(k == 0), stop=(k == 2))
    XWT = ovl.tile([81, BL], f32, tag="big", name="XWT")
    sc.activation(XWT[:], pxw[:], AF.Lrelu,
                  bias=BV[0:81, BV_BFC2 : BV_BFC2 + 1], alpha=0.01)
    ptw = ps_a.tile([BL, 81], f32, tag="pa", name="ptw")
    te.transpose(ptw[:], XWT[:], IDF[0:81, 0:81])
    XWB = ovl.tile([BL, 81], f32, tag="big", name="XWB")
    sc.activation(XWB[:], ptw[:], AF.Copy)

    # linearize [BL, 81] -> b-major rows staged in DRAM; tiles load slices
    lin_scr = []
    for i, srct in enumerate((F22T, SSCC, XWB)):
        cb = wA_pool.tile([BL, 81], bf16, tag="wcb", name=f"cb{i}")
        v.tensor_copy(cb[:], srct[:, :])
        scr = nc.dram_tensor(f"lin_scr{i}", [BL, 81], bf16, kind="Internal")
        nc.sync.dma_start(out=scr.ap(), in_=cb[:, :])
        lin_scr.append(scr)

    stage_f2(0, [3, 4, 5])

    # six per-kc winograd point-value tiles; these reuse the pass-A slots
    # of the "big" tag (every pass-A tile above is dead before these fill)
    val_ck = [ovl.tile([128, NPT, NTL, BL], f16, tag="big", name=f"vck{k}")
              for k in range(KC)]

    for g in range(N_TILES):
        stage_lrows(g)
        if g >= 1 and g + 1 < N_TILES:
            stage_load(g + 1)
        stage_fold_red(g, "r2")
        if g >= 1 and g + 1 < N_TILES:
            stage_f1(g + 1, [0])
        stage_fold_red(g, "r3")
        stage_cor1_q(g)
        if g >= 1 and g + 1 < N_TILES:
            stage_f1(g + 1, [1])
        stage_fold_red(g, "r1")
        if g >= 1 and g + 1 < N_TILES:
            stage_f1(g + 1, [2])
        stage_cor1_bb(g)
        stage_fold_red(g, "r6")
        if g >= 1 and g + 1 < N_TILES:
            stage_f1(g + 1, [3])
        stage_fold_red(g, "r7")
        if g >= 1 and g + 1 < N_TILES:
            stage_f1(g + 1, [4, 5])
        stage_algebra(g)
        stage_fuse21(g)
        if g + 1 < N_TILES:
            stage_f2(g + 1, range(MO))
        stage_cor2(g)
        stage_cor2_bcast(g)
        stage_h(g)
        for i in range(G):
            stage_w1w2(g, i)
    phase2()

def _split_excess_waits(nc, limit=_SYNC_WAIT_LIMIT):
    """walrus allows only a couple of sem waits per instruction; move any
    excess onto same-engine nops inserted right before the instruction."""
    import bass_rust

    cnt = 0
    for f in nc.m.functions:
        for b in f.blocks:
            insts = b.instructions
            newlist = []
            changed = False
            for inst in insts:
                si = getattr(inst, "sync_info", None)
                waits = list(si.on_wait) if si is not None else []
                if len(waits) > limit:
                    changed = True
                    extra, keep = waits[:-limit], waits[-limit:]
                    for j in range(0, len(extra), limit):
                        nop = mybir.InstNoOp(name=f"waitnop_{cnt}", ins=[],
                                             outs=[])
                        cnt += 1
                        nop.engine = inst.engine
                        nop.sync_info = bass_rust.SyncInfo(
                            on_wait=extra[j : j + limit], on_update=[])
                        nc.register_instruction(nop, overwrite=True)
                        newlist.append(nop)
                    inst.sync_info = bass_rust.SyncInfo(
                        on_wait=keep, on_update=list(si.on_update))
                newlist.append(inst)
            if changed:
                insts[:] = newlist


_PROGRAM_CACHE = {}


def _build_program():
    if "nc" in _PROGRAM_CACHE:
        return _PROGRAM_CACHE["nc"]
    _patch_drain_wait_limit()
    nc = bass.Bass("TRN2", target_bir_lowering=False, debug=False,
                   num_devices=1)
    io = {}
    io["xe"] = nc.dram_tensor("xe", [C, NE], bf16, kind="ExternalInput")
    io["ye"] = nc.dram_tensor("ye", [L, NE], bf16, kind="ExternalInput")
    io["wh1"] = nc.dram_tensor("wh1", [C, C], bf16, kind="ExternalInput")
    io["wf2x"] = nc.dram_tensor("wf2x", [C, C], bf16, kind="ExternalInput")
    io["wf2y"] = nc.dram_tensor("wf2y", [L, C], bf16, kind="ExternalInput")
    io["a3x"] = nc.dram_tensor("a3x", [C, 2], bf16, kind="ExternalInput")
    io["sy4"] = nc.dram_tensor("sy4", [L, 4], bf16, kind="ExternalInput")
    io["wg"] = nc.dram_tensor("wg", [MO, NPT, 128, KC, 128], f16,
                              kind="ExternalInput")
    io["tinr"] = nc.dram_tensor("tinr", [81, NPTL], f16, kind="ExternalInput")
    io["tinva"] = nc.dram_tensor("tinva", [128, 81], f16,
                                 kind="ExternalInput")
    io["tinvb"] = nc.dram_tensor("tinvb", [98, 81], f16,
                                 kind="ExternalInput")
    io["bnbrep"] = nc.dram_tensor("bnbrep", [MO, BL * 128], f16,
                                  kind="ExternalInput")
    io["wfc1"] = nc.dram_tensor("wfc1", [81, 324], bf16, kind="ExternalInput")
    io["wfc2"] = nc.dram_tensor("wfc2", [384, 81], bf16, kind="ExternalInput")
    io["shw"] = nc.dram_tensor("shw", [81, 81], f32, kind="ExternalInput")
    io["sb"] = nc.dram_tensor("sb", [BE, BL], f32, kind="ExternalInput")
    io["bv"] = nc.dram_tensor("bv", [128, BV_NCOLS], f32, kind="ExternalInput")
    io["out"] = nc.dram_tensor("out", [81, BL, O], f32, kind="ExternalOutput")

    from contextlib import ExitStack

    with tile.TileContext(nc) as tc, ExitStack() as ctx:
        _emit(ctx, nc, tc, io)
    _split_excess_waits(nc)
    _PROGRAM_CACHE["nc"] = nc
    return nc


def _reflect_filter_1d(n, win):
    """uniform_filter1d with reflect ('symmetric') padding as an n x n map."""
    r = win // 2
    s = np.zeros((n, n), np.float64)
    for o in range(n):
        for k in range(o - r, o + r + 1):
            i = k
            if i < 0:
                i = -i - 1
            if i > n - 1:
                i = 2 * n - 1 - i
            s[o, i] += 1.0 / win
    return s


def host_prepare(inputs):
    f64 = np.float64
    x = np.asarray(inputs["x"], np.float32)
    y = np.asarray(inputs["y"], np.float32)
    W11 = np.asarray(inputs["w_conv1_1"], f64)
    wf2x = (W11[:, :C2] @ np.asarray(inputs["w_convh2"], f64)).astype(np.float32)
    wf2y = (W11[:, C2:] @ np.asarray(inputs["w_convl1"], f64)).astype(np.float32)
    b_f2 = (W11[:, :C2] @ np.asarray(inputs["b_convh2"], f64)
            + W11[:, C2:] @ np.asarray(inputs["b_convl1"], f64)
            + np.asarray(inputs["b_conv1_1"], f64)).astype(np.float32)
    w12 = np.asarray(inputs["w_conv1_2"], f64)
    a3x = (w12[:, 0:1] @ np.asarray(inputs["w_convh3"], f64)).astype(np.float32)
    a3y = (w12[:, 1:2] @ np.asarray(inputs["w_convl2"], f64)).astype(np.float32)
    b3 = (w12 @ np.concatenate([np.asarray(inputs["b_convh3"], f64),
                                np.asarray(inputs["b_convl2"], f64)])
          + np.asarray(inputs["b_conv1_2"], f64)).astype(np.float32)
    bias4 = np.concatenate([b3, np.asarray(inputs["b_convl3"], np.float32)])

    sy4 = np.concatenate(
        [a3y.T, np.asarray(inputs["w_convl3"], np.float32).T], axis=1)

    s1 = _reflect_filter_1d(HH, WIN)
    shw = np.kron(s1, s1).T.astype(np.float32)  # lhsT [in_px, out_px]
    sb_m = np.zeros((BE, BL), np.float32)
    for o in range(BL):
        sb_m[o : o + WIN, o] = 1.0 / WIN

    w_pool = np.asarray(inputs["w_pool"], f64)  # (2, 1, 3, 3)
    mconv = np.zeros((2, 81, 81), f64)          # [c, out_px, in_px]
    for c in range(2):
        for oh in range(HH):
            for ow in range(WW):
                for dh in range(3):
                    for dw in range(3):
                        ih, iw = oh + dh - 1, ow + dw - 1
                        if 0 <= ih < HH and 0 <= iw < WW:
                            mconv[c, oh * WW + ow, ih * WW + iw] = \
                                w_pool[c, 0, dh, dw]

    bfd = ml_dtypes.bfloat16
    W1 = np.asarray(inputs["w_fc1"], f64)
    bp = np.asarray(inputs["b_pool"], f64)
    wf = (W1[:, 0:81] + W1[:, 243:324]
          + W1[:, 81:162] @ mconv[0] + W1[:, 162:243] @ mconv[1])
    wfc1 = np.ascontiguousarray(wf.T).astype(bfd)       # lhsT [81, 324]
    bfc1 = (np.asarray(inputs["b_fc1"], f64)
            + bp[0] * W1[:, 81:162].sum(axis=1)
            + bp[1] * W1[:, 162:243].sum(axis=1)).astype(np.float32)
    wfc2 = np.zeros((384, 81), bfd)
    wfc2[:324] = np.asarray(inputs["w_fc2"], np.float32).T.astype(bfd)

    bn_scale = (np.asarray(inputs["bn_gamma"], f64)
                / np.sqrt(np.asarray(inputs["bn_var"], f64) + 1e-5))
    bn_bias = (np.asarray(inputs["bn_beta"], f64)
               - np.asarray(inputs["bn_mean"], f64) * bn_scale)

    bv = np.zeros((128, BV_NCOLS), np.float32)
    b_h1 = np.asarray(inputs["b_convh1"], np.float32)
    for m in range(MO):
        bv[:, BV_BH1 + m] = b_h1[m * 128 : (m + 1) * 128]
        bv[:, BV_BF2 + m] = b_f2[m * 128 : (m + 1) * 128]
        bv[:, BV_BNS + m] = bn_scale[m * 128 : (m + 1) * 128]
        bv[:, BV_BNB + m] = bn_bias[m * 128 : (m + 1) * 128]
    bv[0:2, BV_B4] = bias4[0:2]
    bv[0:2, BV_B4Y] = bias4[2:4]
    for mo in range(3):
        osz = min(128, 324 - mo * 128)
        bv[0:osz, BV_BFC1 + mo] = bfc1[mo * 128 : mo * 128 + osz]
    bv[0:81, BV_BFC2] = np.asarray(inputs["b_fc2"], np.float32)
    bv[:, BV_W00] = np.float32(inputs["w_cc1"][0, 0])
    bv[:, BV_W01] = np.float32(inputs["w_cc1"][0, 1])
    bv[:, BV_BCC] = np.float32(inputs["b_cc1"][0])
    bv[:, BV_BP0] = np.float32(inputs["b_pool"][0])
    bv[:, BV_BP1] = np.float32(inputs["b_pool"][1])

    # winograd transforms + transformed conv weights (bn scale folded in)
    f16n = np.float16
    Gm, Tin, Tinv = _wino_transforms()
    tinr = Tin.T.astype(f16n)                      # [81, 225] rhs
    tinva = Tinv[:, 0:128].T.astype(f16n)          # [128, 81] lhsT chunk A
    tinvb = np.ones((98, 81), np.float64)
    tinvb[0:97] = Tinv[:, 128:NPTL].T              # row 97 stays 1.0 (bias)
    tinvb = tinvb.astype(f16n)
    wb_s = (np.asarray(inputs["w_bconv"], f64)
            * bn_scale[:, None, None, None])       # fold BN scale into conv
    Wg = np.einsum("pi,ocij,qj->pqoc", Gm, wb_s, Gm).reshape(NPT, O, C)
    wg = (Wg.reshape(NPT, MO, 128, KC, 128)
          .transpose(1, 0, 4, 3, 2)).astype(f16n)  # [mo, pt, cp, kc, ocol]
    bnbrep = np.tile(bn_bias.reshape(MO, 1, 128),
                     (1, BL, 1)).reshape(MO, BL * 128).astype(f16n)

    bf = ml_dtypes.bfloat16
    common = {
        "wh1": np.asarray(inputs["w_convh1"], np.float32).T.astype(bf),
        "wf2x": wf2x.T.astype(bf),
        "wf2y": wf2y.T.astype(bf),
        "a3x": a3x.T.astype(bf),
        "sy4": sy4.astype(bf),
        "wg": wg, "tinr": tinr, "tinva": tinva, "tinvb": tinvb,
        "bnbrep": bnbrep,
        "wfc1": wfc1, "wfc2": wfc2,
        "shw": shw, "sb": sb_m, "bv": bv,
    }
    common = {k: np.ascontiguousarray(v) for k, v in common.items()}

    xp = np.pad(x, ((HALO, HALO), (0, 0), (0, 0), (0, 0)), mode="symmetric")
    yp = np.pad(y, ((HALO, HALO), (0, 0), (0, 0), (0, 0)), mode="symmetric")
    in_maps = []
    for m in range(M_CORES):
        xe = np.ascontiguousarray(
            xp[m * BL : m * BL + BE].transpose(1, 0, 2, 3).reshape(C, NE)
        ).astype(bf)
        ye = np.ascontiguousarray(
            yp[m * BL : m * BL + BE].transpose(1, 0, 2, 3).reshape(L, NE)
        ).astype(bf)
        in_maps.append({"xe": xe, "ye": ye, **common})
    return in_maps


def kernel(**inputs):
    nc = _build_program()
    in_maps = host_prepare(inputs)
    trace = os.environ.get("KERNEL_TRACE", "0") == "1"
    kw = {}
    if trace:
        kw = dict(trace=True, trace_cores=[0])
    res = run_bass_kernel_spmd(nc, in_maps, core_ids=list(range(M_CORES)), **kw)
    if trace:
        kernel.last_results = res
        if res.exec_time_ns is not None:
            print(f"HW exec time: {res.exec_time_ns} ns")
    out = np.empty((B, O, HH, WW), np.float32)
    for m in range(M_CORES):
        o = res.results[m]["out"]          # [81, BL, O] pixel-major
        out[m * BL : (m + 1) * BL] = (
            o.reshape(HH, WW, BL, O).transpose(2, 3, 0, 1))
    return out



# revision 44
# speedup vs baseline: 1.2042x; 1.2042x over previous
"""Trainium2 Bass kernel for nn_FAFMoudle (dense_cnn).

Data-parallel across 8 NeuronCores: 32 images per core plus a 3-image halo
on each side for the SSIM uniform filter (which smooths across the batch
axis).  The halo is materialized on the host by symmetrically padding the
global batch, so every core runs an identical program on its own shard.

Device-side plan (per core, all 1x1 convs folded on host into single
matmuls, channel-major layout [C, b*81]):
  pass A: fuse_3/fuse_4 (2ch maps) over the 38 ext images -> SSIM via
          small filter-matrix matmuls (hw-filter 81x81, batch-filter 38x32)
          with PE transposes between; fuse2_2 / cc1(ssim) / xweight
          (fc1+gelu+fc2+leakyrelu) -> linearized per-pixel scalar rows.
  pass B: per 6-image tile: fuse_1/fuse_2 (bf16 matmuls), cosine sims via
          pointwise products + ones-vector PE reductions, fuse2_1/fuse3_1
          chain, xout written into a zero-padded per-image buffer, then the
          3x3 conv as 9*6 accumulating matmuls per output chunk, fused
          BN+leaky-relu on evacuation.
"""

import os
import sys

for _p in (
    "/opt/trn_rl_repo",
    "/root/.axon_site",
    "/root/.axon_site/_ro/trn_rl_repo",
    "/root/.axon_site/_ro/pypackages",
):
    if os.path.isdir(_p) and _p not in sys.path:
        sys.path.insert(0, _p)

import math

import ml_dtypes
import numpy as np

import concourse.bass as bass
import concourse.tile as tile
from concourse import mybir
from concourse.bass_utils import run_bass_kernel_spmd
from concourse.masks import make_identity

dt = mybir.dt
AF = mybir.ActivationFunctionType
ALU = mybir.AluOpType

# ----------------------------------------------------------------------------
# shapes
B, C, L, O, HH, WW = 256, 768, 64, 768, 9, 9
C2, C3 = 2 * C // 3, C // 3
M_CORES = 8
BL = B // M_CORES          # 32 images per core
HALO = 3
BE = BL + 2 * HALO         # 38 ext images
PX = HH * WW               # 81
NV = BL * PX               # 2592 valid pixels
NE = BE * PX               # 3078 ext pixels
KC = C // 128              # 6 contraction chunks
MO = O // 128              # 6 output chunks
G = 6                      # images per pass-B tile
TW = G * PX                # 486
N_TILES = (BL + G - 1) // G
WIN = 7
COV = (WIN ** 3) / (WIN ** 3 - 1.0)
C1S, C2S = 0.01 ** 2, 0.03 ** 2
SQRT_C = math.sqrt(C)
# padded per-image layout for the 3x3 conv input: 11 rows x 12 cols,
# interior at rows 1..9, cols 2..10 (keeps every 9-wide run 4B aligned)
IMR, IMC = 11, 12
IMS = IMR * IMC            # 132

bf16 = dt.bfloat16
f32 = dt.float32
f16 = dt.float16

NPT = 25                   # winograd F(3,3) points per tile (5x5)
NTL = 9                    # 3x3 output tiles per 9x9 image
NPTL = NPT * NTL           # 225 (pt, tile) pairs


def _wino_transforms():
    """F(3,3) 2D Winograd with points {0,1,-1,2} + inf, zero-padding folded
    into the input transform.  Row order: ptile = pt*9 + tile."""
    pts = [0.0, 1.0, -1.0, 2.0]
    V = np.zeros((5, 5))
    V3 = np.zeros((5, 3))
    for i, p in enumerate(pts):
        V[i] = [p ** j for j in range(5)]
        V3[i] = [p ** j for j in range(3)]
    V[4] = [0, 0, 0, 0, 1]
    V3[4] = [0, 0, 1]
    AT = V3.T                      # 3x5
    Gm = V3                        # 5x3 (kernel transform)
    BT = np.linalg.inv(V).T        # 5x5 (input transform)
    Tin = np.zeros((NPTL, 81))
    Tinv = np.zeros((81, NPTL))
    for ti in range(3):
        for tj in range(3):
            tl = ti * 3 + tj
            for pi in range(5):
                for pj in range(5):
                    row = (pi * 5 + pj) * NTL + tl
                    for a in range(5):
                        for b in range(5):
                            r, c = 3 * ti + a - 1, 3 * tj + b - 1
                            if 0 <= r < 9 and 0 <= c < 9:
                                Tin[row, r * 9 + c] += BT[pi, a] * BT[pj, b]
            for oi in range(3):
                for oj in range(3):
                    orow = (3 * ti + oi) * 9 + (3 * tj + oj)
                    for pi in range(5):
                        for pj in range(5):
                            Tinv[orow, (pi * 5 + pj) * NTL + tl] = \
                                AT[oi, pi] * AT[oj, pj]
    return Gm, Tin, Tinv

# BV (bias/const matrix) column map
BV_BH1 = 0          # 6 cols
BV_BF2 = 6          # 6 cols
BV_B4 = 12          # 1 col (rows 0:2, f3 bias)
BV_BFC1 = 13        # 3 cols
BV_BFC2 = 16        # 1 col (rows 0:81)
BV_BNS = 17         # 6 cols
BV_BNB = 23         # 6 cols
BV_W00 = 29
BV_W01 = 30
BV_BCC = 31
BV_BP0 = 32
BV_BP1 = 33
BV_B4Y = 34         # f4 bias (rows 0:2)
BV_NCOLS = 35

_SYNC_WAIT_LIMIT = 1


def _patch_drain_wait_limit():
    """walrus in this container only allows 2 sem waits per TPB_CTRL
    instruction; split the tile-exit drain's waits across extra nops."""
    import bass_rust
    from concourse.tile import ScopedClock, TileContext

    if getattr(TileContext, "_drain_waits_patched", False):
        return

    def _drain_and_barrier(self, tick_clock, wait_clock):
        drain_inst = self.nc.sync.drain()
        wait_clock.add_sem_waits(
            drain_inst.ins, ScopedClock({None: tick_clock.global_clock})
        )
        si = drain_inst.ins.sync_info
        waits = list(si.on_wait)
        if len(waits) > _SYNC_WAIT_LIMIT:
            drain_inst.ins.sync_info = bass_rust.SyncInfo(
                on_wait=waits[:_SYNC_WAIT_LIMIT], on_update=list(si.on_update)
            )
            for i in range(_SYNC_WAIT_LIMIT, len(waits), _SYNC_WAIT_LIMIT):
                n = self.nc.sync.nop()
                n.ins.sync_info = bass_rust.SyncInfo(
                    on_wait=waits[i : i + _SYNC_WAIT_LIMIT], on_update=[]
                )
        self.nc.all_engine_barrier()
        popped = self.nc._tile_sem_poison_stack.pop()
        assert popped is self._sem_poison
        self.nc.clear_and_free_semaphores(list(self.sems.allocated().values()))
        self.nc.all_engine_barrier()

    TileContext._drain_and_barrier = _drain_and_barrier
    TileContext._drain_waits_patched = True


def _emit(ctx, nc, tc, io):
    v = nc.vector
    sc = nc.scalar
    te = nc.tensor

    cp = ctx.enter_context(tc.tile_pool(name="const", bufs=1))
    # "big" tag: pass-A persistent tiles share six 14.4KB slots with the six
    # per-kc winograd point-value tiles (val_ck) that only start filling in
    # pass B, after every pass-A tile is dead
    ovl = ctx.enter_context(tc.tile_pool(name="ovl", bufs=6))
    xt_pool = ctx.enter_context(tc.tile_pool(name="xt", bufs=2))
    f_pool = ctx.enter_context(tc.tile_pool(name="fs", bufs=2))
    prod_pool = ctx.enter_context(tc.tile_pool(name="prod", bufs=4))
    bcs_pool = ctx.enter_context(tc.tile_pool(name="bcs", bufs=2))
    sc_pool = ctx.enter_context(tc.tile_pool(name="sct", bufs=3))
    out_pool = ctx.enter_context(tc.tile_pool(name="outp", bufs=2))
    wA_pool = ctx.enter_context(tc.tile_pool(name="wA", bufs=1))
    ht_pool = ctx.enter_context(tc.tile_pool(name="ht", bufs=2))
    wg_pool = ctx.enter_context(tc.tile_pool(name="wgp", bufs=4))
    vo_pool = ctx.enter_context(tc.tile_pool(name="vo", bufs=1))
    vp_pool = ctx.enter_context(tc.tile_pool(name="vp", bufs=1))

    ps_a = ctx.enter_context(tc.tile_pool(name="psA", bufs=3, space="PSUM"))
    ps_red = ctx.enter_context(tc.tile_pool(name="psRed", bufs=1, space="PSUM"))

    # ---- constants / weights into SBUF --------------------------------
    def ld(name, shape, dtype, ap):
        t = cp.tile(shape, dtype, name=name)
        nc.sync.dma_start(out=t[:], in_=ap)
        return t

    A3X = ld("A3X", [128, KC, 2], bf16,
             io["a3x"].ap().rearrange("(kc p) m -> p kc m", p=128))
    SY4 = ld("SY4", [L, 4], bf16, io["sy4"].ap())
    BV = ld("BV", [128, BV_NCOLS], f32, io["bv"].ap())
    ye_ap = io["ye"].ap()
    xe_re0 = io["xe"].ap().rearrange("(kc p) n -> p kc n", p=128)

    # first pass-A chunk DMAs go out before the bulky consts so the PE can
    # start as soon as possible
    preA = {}

    def loadA(c0, w):
        xa = xt_pool.tile([128, KC, TW], bf16, tag="xt", name=f"xa{c0}")
        nc.sync.dma_start(out=xa[:, :, :w], in_=xe_re0[:, :, c0 : c0 + w])
        ya = xt_pool.tile([L, TW], bf16, tag="yt", name=f"ya{c0}")
        nc.sync.dma_start(out=ya[:, :w], in_=ye_ap[:, c0 : c0 + w])
        return xa, ya

    preA[0] = loadA(0, min(TW, NE))

    WFC1 = ld("WFC1", [81, 324], bf16, io["wfc1"].ap())
    WFC2 = ld("WFC2", [128, 3, 81], bf16,
              io["wfc2"].ap().rearrange("(kc p) m -> p kc m", p=128))
    SHW = ld("SHW", [81, 81], f32, io["shw"].ap())
    SB = ld("SB", [BE, BL], f32, io["sb"].ap())
    TINR = ld("TINR", [81, NPTL], f16, io["tinr"].ap())
    TINVA = ld("TINVA", [128, 81], f16, io["tinva"].ap())
    TINVB = ld("TINVB", [98, 81], f16, io["tinvb"].ap())

    IDF = cp.tile([128, 128], f32, name="IDF")
    make_identity(nc, IDF[:])
    IDFB = cp.tile([128, 128], bf16, name="IDFB")
    make_identity(nc, IDFB[:])
    IDFH = cp.tile([128, 128], f16, name="IDFH")
    make_identity(nc, IDFH[:])
    ONESC = cp.tile([128, 1], bf16, name="ONESC")
    nc.gpsimd.memset(ONESC[:], 1.0)
    ONESR = cp.tile([1, 128], bf16, name="ONESR")
    nc.gpsimd.memset(ONESR[:], 1.0)
    EPSR = cp.tile([1, 1], f32, name="EPSR")
    nc.gpsimd.memset(EPSR[:], 1e-16)

    xe_re = io["xe"].ap().rearrange("(kc p) n -> p kc n", p=128)

    st = {}

    def tdims(g):
        gi = min(G, BL - g * G)
        return gi, gi * PX, g * TW, HALO * PX + g * TW

    def stage_load(g):
        gi, w, c0, ce = tdims(g)
        xt = xt_pool.tile([128, KC, TW], bf16, tag="xt", name=f"xt{g}")
        nc.sync.dma_start(out=xt[:, :, :w], in_=xe_re[:, :, ce : ce + w])
        yt = xt_pool.tile([L, TW], bf16, tag="yt", name=f"yt{g}")
        nc.sync.dma_start(out=yt[:, :w], in_=ye_ap[:, ce : ce + w])
        st[g] = {"xt": xt, "yt": yt}

    def stage_f1(g, ms):
        gi, w, c0, ce = tdims(g)
        s = st[g]
        if "F1S" not in s:
            s["F1S"] = f_pool.tile([128, KC, TW], bf16, tag="f1s",
                                   name=f"f1s{g}")
        F1S = s["F1S"]
        for m in ms:
            p1 = ps_a.tile([128, TW], f32, tag="pa", name=f"p1_{g}_{m}")
            for k in range(KC):
                te.matmul(p1[:, :w], WH1[:, k, m * 128 : (m + 1) * 128],
                          s["xt"][:, k, :w], start=(k == 0),
                          stop=(k == KC - 1))
            sc.activation(F1S[:, m, :w], p1[:, :w], AF.Identity,
                          bias=BV[:, BV_BH1 + m : BV_BH1 + m + 1])

    def stage_f2(g, ms):
        gi, w, c0, ce = tdims(g)
        s = st[g]
        if "F2S" not in s:
            s["F2S"] = f_pool.tile([128, KC, TW], bf16, tag="f2s", bufs=2,
                                   name=f"f2s{g}")
        F2S = s["F2S"]
        for m in ms:
            p2 = ps_a.tile([128, TW], f32, tag="pa", name=f"p2_{g}_{m}")
            te.matmul(p2[:, :w], WF2Y[:, m * 128 : (m + 1) * 128],
                      s["yt"][:, :w], start=True, stop=False)
            for k in range(KC):
                te.matmul(p2[:, :w], WF2X[:, k, m * 128 : (m + 1) * 128],
                          s["xt"][:, k, :w], start=False, stop=(k == KC - 1))
            sc.activation(F2S[:, m, :w], p2[:, :w], AF.Identity,
                          bias=BV[:, BV_BF2 + m : BV_BF2 + m + 1])

    def stage_fold_red(g, which):
        # 6-fold the channel-chunk terms on DVE, then one M=1 matmul into
        # a packed psum row (rows 32-aligned so groups stay independent)
        gi, w, c0, ce = tdims(g)
        s = st[g]
        F1S, F2S = s["F1S"], s["F2S"]
        if "rr" not in s:
            s["rr"] = ps_red.tile([1, 5 * 512], f32, tag="red", name=f"rr_{g}")
        spec = {
            "r1": (0, F1S, F2S),
            "r2": (1, F1S, F1S),
            "r3": (2, F2S, F2S),
            "r6": (3, F1S, None),
            "r7": (4, F2S, None),
        }
        slot, a, b = spec[which]
        eng = nc.gpsimd if which in ("r2", "r3", "r6", "r7") else v
        rt = s["rr"]
        acc = prod_pool.tile([128, TW], bf16, tag="pp", name=f"ac{which}{g}")
        if b is None:
            eng.tensor_add(acc[:, :w], a[:, 0, :w], a[:, 1, :w])
            for m in range(2, MO):
                eng.tensor_add(acc[:, :w], acc[:, :w], a[:, m, :w])
        else:
            eng.tensor_mul(acc[:, :w], a[:, 0, :w], b[:, 0, :w])
            for m in range(1, MO):
                tmp = prod_pool.tile([128, TW], bf16, tag="pp",
                                     name=f"tp{which}{g}_{m}")
                eng.tensor_mul(tmp[:, :w], a[:, m, :w], b[:, m, :w])
                eng.tensor_add(acc[:, :w], acc[:, :w], tmp[:, :w])
        te.matmul(rt[0:1, 512 * slot : 512 * slot + w], ONESC[:],
                  acc[:, :w], start=True, stop=True)

    def stage_cor1_q(g):
        # issued right after the r2/r3 folds: the 1/sqrt(r2*r3) chain runs
        # on scalar while the r1 fold is still going on vector
        gi, w, c0, ce = tdims(g)
        s = st[g]
        rr = s["rr"]
        r2 = rr[0:1, 512 : 512 + TW]
        r3 = rr[0:1, 1024 : 1024 + TW]
        q1 = sc_pool.tile([1, TW], f32, tag="scf", bufs=4, name=f"q1_{g}")
        q3 = sc_pool.tile([1, TW], f32, tag="scf", bufs=4, name=f"q3_{g}")
        qs = sc_pool.tile([1, TW], f32, tag="scf", bufs=4, name=f"qs_{g}")
        sc.activation(q3[:, :w], r3[:, :w], AF.Copy)
        v.tensor_mul(qs[:, :w], r2[:, :w], q3[:, :w])
        # 1/sqrt(x) as exp(-0.5*ln(x)) -- keeps the whole chain on the scalar
        # engine instead of DVE's ~3.8us iterative reciprocal
        sc.activation(qs[:, :w], qs[:, :w], AF.Ln, bias=EPSR[0:1, 0:1])
        sc.activation(q1[:, :w], qs[:, :w], AF.Exp, scale=-0.5)
        s["q1"] = q1

    def stage_cor1_bb(g):
        gi, w, c0, ce = tdims(g)
        s = st[g]
        rr = s["rr"]
        r1 = rr[0:1, 0:TW]
        q1 = s["q1"]
        beta = sc_pool.tile([1, TW], bf16, tag="scb", name=f"beta{g}")
        q2 = sc_pool.tile([1, TW], f32, tag="scf", bufs=4, name=f"q2_{g}")
        v.scalar_tensor_tensor(q2[:, :w], r1[:, :w], -0.5, q1[:, :w],
                               ALU.mult, ALU.mult)
        v.tensor_scalar_add(beta[:, :w], q2[:, :w], 0.5)
        s["beta"] = beta
        bb = ps_a.tile([128, TW], f32, tag="pa", name=f"bb{g}")
        te.matmul(bb[:, :w], ONESR[:], beta[:, :w], start=True, stop=True)
        bbs = bcs_pool.tile([128, TW], bf16, tag="bcs", name=f"bbs{g}")
        sc.activation(bbs[:, :w], bb[:, :w], AF.Copy)
        s["bbs"] = bbs

    def stage_algebra(g):
        gi, w, c0, ce = tdims(g)
        s = st[g]
        rr = s["rr"]
        r1 = rr[0:1, 0:TW]
        r2 = rr[0:1, 512 : 512 + TW]
        r3 = rr[0:1, 1024 : 1024 + TW]
        r6 = rr[0:1, 1536 : 1536 + TW]
        r7 = rr[0:1, 2048 : 2048 + TW]
        beta = s["beta"]
        # r4 = r6 + beta*r7   (fuse2_1 channel-sum, no extra reduction)
        r4s = sc_pool.tile([1, TW], f32, tag="scf", bufs=4, name=f"r4s_{g}")
        v.tensor_mul(r4s[:, :w], beta[:, :w], r7[:, :w])
        v.tensor_add(r4s[:, :w], r4s[:, :w], r6[:, :w])
        s["r4s"] = r4s
        # r5 = r2 + 2*beta*r1 + beta^2*r3
        t1 = sc_pool.tile([1, TW], f32, tag="scf", bufs=4, name=f"t1_{g}")
        t2 = sc_pool.tile([1, TW], f32, tag="scf", bufs=4, name=f"t2_{g}")
        v.tensor_mul(t1[:, :w], beta[:, :w], r1[:, :w])
        v.tensor_mul(t2[:, :w], beta[:, :w], r3[:, :w])
        v.tensor_mul(t2[:, :w], beta[:, :w], t2[:, :w])
        v.scalar_tensor_tensor(t1[:, :w], t1[:, :w], 2.0, t2[:, :w],
                               ALU.mult, ALU.add)
        v.tensor_add(t1[:, :w], t1[:, :w], r2[:, :w])
        s["r5s"] = t1

    def stage_fuse21(g):
        gi, w, c0, ce = tdims(g)
        s = st[g]
        F1S, F2S, bbs = s["F1S"], s["F2S"], s["bbs"]
        for m in range(MO):
            td = prod_pool.tile([128, TW], bf16, tag="pp", name=f"td{g}_{m}")
            v.tensor_mul(td[:, :w], bbs[:, :w], F2S[:, m, :w])
            # fuse2_1 overwrites F1S in place
            v.tensor_add(F1S[:, m, :w], td[:, :w], F1S[:, m, :w])

    def stage_lrows(g):
        gi, w, c0, ce = tdims(g)
        s = st[g]
        lr = sc_pool.tile([1, 3, TW], bf16, tag="lrow", bufs=2,
                          name=f"lr{g}")
        for nm_, idx in (("f22l", 0), ("sccl", 1), ("xwl", 2)):
            nc.sync.dma_start(
                out=lr[0:1, idx, :w],
                in_=lin_scr[idx].ap().rearrange(
                    "(one b) q -> one (b q)", one=1)[:, c0 : c0 + w])
        s["f22l"] = lr[0:1, 0, :]
        s["sccl"] = lr[0:1, 1, :]
        s["xwl"] = lr[0:1, 2, :]

    def stage_cor2(g):
        gi, w, c0, ce = tdims(g)
        s = st[g]
        r4s, r5s = s["r4s"], s["r5s"]
        f22l = s["f22l"]
        nmr = sc_pool.tile([1, TW], f32, tag="scf", bufs=4, name=f"nm{g}")
        v.tensor_mul(nmr[:, :w], f22l[:, :w], r4s[:, :w])
        # 1/(sqrt(r5)*|f22l|*sqrt(C)) = exp(-0.5*ln(r5*f22l^2*C))
        s5 = sc_pool.tile([1, TW], f32, tag="scf", bufs=4, name=f"s5_{g}")
        af_ = sc_pool.tile([1, TW], f32, tag="scf", bufs=4, name=f"af{g}")
        v.tensor_mul(af_[:, :w], f22l[:, :w], f22l[:, :w])
        v.tensor_mul(s5[:, :w], r5s[:, :w], af_[:, :w])
        sc.activation(s5[:, :w], s5[:, :w], AF.Ln, scale=float(C), bias=EPSR[0:1, 0:1])
        s5i = sc_pool.tile([1, TW], f32, tag="scf", bufs=4, name=f"s5i_{g}")
        sc.activation(s5i[:, :w], s5[:, :w], AF.Exp, scale=-0.5)
        v.tensor_mul(nmr[:, :w], nmr[:, :w], s5i[:, :w])    # cor2
        v.tensor_sub(nmr[:, :w], nmr[:, :w], s["sccl"][:, :w])
        v.tensor_scalar(nmr[:, :w], nmr[:, :w], -0.5, 0.5, ALU.mult, ALU.add)
        delta = sc_pool.tile([1, TW], bf16, tag="scb", name=f"dl{g}")
        v.tensor_mul(delta[:, :w], nmr[:, :w], f22l[:, :w])
        s["delta"] = delta
        xw1 = sc_pool.tile([1, TW], bf16, tag="scb", name=f"xw1_{g}")
        v.tensor_scalar_add(xw1[:, :w], s["xwl"][:, :w], 1.0)
        s["xw1"] = xw1

    def stage_cor2_bcast(g):
        gi, w, c0, ce = tdims(g)
        s = st[g]
        bd = ps_a.tile([128, TW], f32, tag="pa", name=f"bd{g}")
        te.matmul(bd[:, :w], ONESR[:], s["delta"][:, :w], start=True,
                  stop=True)
        dbs = bcs_pool.tile([128, TW], bf16, tag="bcs", name=f"dbs{g}")
        sc.activation(dbs[:, :w], bd[:, :w], AF.Copy)
        s["dbs"] = dbs
        bw = ps_a.tile([128, TW], f32, tag="pa", name=f"bw{g}")
        te.matmul(bw[:, :w], ONESR[:], s["xw1"][:, :w], start=True, stop=True)
        wbs = bcs_pool.tile([128, TW], bf16, tag="bcs", name=f"wbs{g}")
        sc.activation(wbs[:, :w], bw[:, :w], AF.Copy)
        s["wbs"] = wbs

    def stage_h(g):
        # h = (fuse2_1 + delta_bcast) * (1 + xweight)_bcast, in place in F1S
        gi, w, c0, ce = tdims(g)
        s = st[g]
        F1S, dbs, wbs = s["F1S"], s["dbs"], s["wbs"]
        for m in range(MO):
            eng = v if m % 2 == 0 else nc.gpsimd
            eng.tensor_add(F1S[:, m, :w], F1S[:, m, :w], dbs[:, :w])
            eng.tensor_mul(F1S[:, m, :w], F1S[:, m, :w], wbs[:, :w])

    def stage_w1w2(g, i):
        # one image: transpose h to pixel-major, then the fused winograd
        # input transform with the image data as lhsT -> val_ck channel-major
        gi, w, c0, ce = tdims(g)
        if i >= gi:
            return
        s = st[g]
        F1S = s["F1S"]
        im = g * G + i
        HT = ht_pool.tile([81, KC, 128], f16, tag="ht", name=f"ht{g}_{i}")
        for half in range(2):
            tp = ps_a.tile([81, 3 * 128], bf16, tag="pa",
                           name=f"w1p{g}_{i}_{half}")
            for k in range(3):
                m = half * 3 + k
                te.transpose(tp[:, k * 128 : (k + 1) * 128],
                             F1S[:, m, i * PX : i * PX + PX], IDFB[:])
            sc.activation(HT[:, half * 3 : half * 3 + 3, :],
                          tp[:].rearrange("p (k c) -> p k c", c=128), AF.Copy)
        for k in range(KC):
            tq = ps_a.tile([128, NPTL], f32, tag="pa", name=f"w2p{g}_{i}_{k}")
            te.matmul(tq[:], HT[:, k, :], TINR[:], start=True, stop=True)
            if k < 3:
                sc.activation(val_ck[k][:, :, :, im],
                              tq[:].rearrange("p (pt t) -> p pt t", t=NTL),
                              AF.Copy)
            else:
                v.tensor_copy(val_ck[k][:, :, :, im],
                              tq[:].rearrange("p (pt t) -> p pt t", t=NTL))

    def phase2():
        wg_re = io["wg"].ap()         # [MO, 25, 128, KC, 128]
        wg_fifo = []
        bnb_re = io["bnbrep"].ap()    # [MO, BL*128]
        out2_re = io["out"].ap()      # [81, BL, O]
        FW = BL * 128                 # 4096 inverse-transform columns per mo
        CHW = 384                     # 3 images per chunk (psum-bank sized)
        nch = (FW + CHW - 1) // CHW
        for mo in range(MO):
            vo = vo_pool.tile([128, NPT, NTL, BL], f16, tag="vo",
                              name=f"vo{mo}")
            # keep 3 point-weight DMAs in flight ahead of the PE
            for pt in range(NPT):
                if pt == 0:
                    for pf in range(3):
                        wgt_ = wg_pool.tile([128, KC, 128], f16, tag="wg",
                                            name=f"wg{mo}_{pf}")
                        nc.sync.dma_start(out=wgt_[:], in_=wg_re[mo, pf])
                        wg_fifo.append(wgt_)
                if pt + 3 < NPT:
                    wgt_ = wg_pool.tile([128, KC, 128], f16, tag="wg",
                                        name=f"wg{mo}_{pt + 3}")
                    nc.sync.dma_start(out=wgt_[:], in_=wg_re[mo, pt + 3])
                    wg_fifo.append(wgt_)
                wgt = wg_fifo.pop(0)
                pq = ps_a.tile([128, NTL * BL], f32, tag="pa",
                               name=f"pq{mo}_{pt}")
                for k in range(KC):
                    te.matmul(pq[:], wgt[:, k, :], val_ck[k][:, pt, :, :],
                              start=(k == 0), stop=(k == KC - 1))
                if pt % 2 == 0:
                    sc.activation(
                        vo[:, pt, :, :],
                        pq[:].rearrange("p (t b) -> p t b", b=BL), AF.Copy)
                else:
                    v.tensor_copy(
                        vo[:, pt, :, :],
                        pq[:].rearrange("p (t b) -> p t b", b=BL))
            HB = BL // 2
            for hf in range(2):
                vpA = vp_pool.tile([128, HB, 128], f16, tag="vpa",
                                   name=f"vpa{mo}_{hf}")
                vpB = vp_pool.tile([98, HB, 128], f16, tag="vpb",
                                   name=f"vpb{mo}_{hf}")
                nc.sync.dma_start(
                    out=vpB[97:98, :, :].rearrange("o b c -> o (b c)"),
                    in_=bnb_re[mo : mo + 1,
                               hf * HB * 128 : (hf + 1) * HB * 128])
                for i in range(HB):
                    im = hf * HB + i
                    va = vo[:, :, :, im].rearrange("p pt t -> p (pt t)")
                    t5a = ps_a.tile([128, 128], f16, tag="pa",
                                    name=f"t5a{mo}_{im}")
                    te.transpose(t5a[:], va[:, 0:128], IDFH[:])
                    sc.activation(vpA[:, i, :], t5a[:], AF.Copy)
                    t5b = ps_a.tile([97, 128], f16, tag="pa",
                                    name=f"t5b{mo}_{im}")
                    te.transpose(t5b[:], va[:, 128:NPTL], IDFH[:])
                    v.tensor_copy(vpB[0:97, i, :], t5b[:])
                vaf = vpA[:].rearrange("p b c -> p (b c)")
                vbf = vpB[:].rearrange("p b c -> p (b c)")
                FWH = HB * 128
                nch = (FWH + CHW - 1) // CHW
                for ch in range(nch):
                    f0 = ch * CHW
                    fw = min(CHW, FWH - f0)
                    tv = ps_a.tile([81, CHW], f32, tag="pa",
                                   name=f"ti{mo}_{hf}_{ch}")
                    te.matmul(tv[:, :fw], TINVA[:], vaf[:, f0 : f0 + fw],
                              start=True, stop=False)
                    te.matmul(tv[:, :fw], TINVB[:], vbf[:, f0 : f0 + fw],
                              start=False, stop=True)
                    ob = out_pool.tile([81, CHW], f32, tag="ot",
                                       name=f"ob{mo}_{hf}_{ch}")
                    sc.activation(ob[:, :fw], tv[:, :fw], AF.Lrelu,
                                  alpha=0.01)
                    b0 = hf * HB + 3 * ch
                    nc.sync.dma_start(
                        out=out2_re[:, b0 : b0 + fw // 128,
                                    mo * 128 : (mo + 1) * 128],
                        in_=ob[:, :fw].rearrange("p (b c) -> p b c", c=128))


    # =========================== pass A ================================
    # fuse_3 / fuse_4 over ext pixels, transposed per image into
    # T34 [81, (t, b)] with t in {f3c0, f3c1, f4c0, f4c1}
    T34 = ovl.tile([81, 4, BE], f32, tag="big", name="T34")
    chunksA = [(c0, min(TW, NE - c0)) for c0 in range(0, NE, TW)]
    for c0, w in chunksA:
        nb = w // PX
        b0 = c0 // PX
        if c0 in preA:
            xa, ya = preA.pop(c0)
        else:
            xa, ya = loadA(c0, w)
        f3p = ps_a.tile([2, TW], f32, tag="pa", name=f"f3p{c0}")
        f4p = ps_a.tile([2, TW], f32, tag="pa", name=f"f4p{c0}")
        te.matmul(f4p[:, :w], SY4[:, 2:4], ya[:, :w], start=True, stop=True)
        te.matmul(f3p[:, :w], SY4[:, 0:2], ya[:, :w], start=True, stop=False)
        for k in range(KC):
            te.matmul(f3p[:, :w], A3X[:, k, :], xa[:, k, :w],
                      start=False, stop=(k == KC - 1))
        f3s = xt_pool.tile([2, TW], f32, tag="f3s", bufs=1, name=f"f3s{c0}")
        f4s = xt_pool.tile([2, TW], f32, tag="f4s", bufs=1, name=f"f4s{c0}")
        sc.activation(f3s[:, :w], f3p[:, :w], AF.Identity,
                      bias=BV[0:2, BV_B4 : BV_B4 + 1])
        sc.activation(f4s[:, :w], f4p[:, :w], AF.Identity,
                      bias=BV[0:2, BV_B4Y : BV_B4Y + 1])
        pt = ps_a.tile([81, 4 * G], f32, tag="pa", name=f"pt{c0}")
        for i in range(nb):
            te.transpose(pt[:, 4 * i : 4 * i + 2],
                         f3s[:, i * 81 : (i + 1) * 81], IDF[0:2, 0:2])
            te.transpose(pt[:, 4 * i + 2 : 4 * i + 4],
                         f4s[:, i * 81 : (i + 1) * 81], IDF[0:2, 0:2])
        sc.activation(
            T34[:, :, b0 : b0 + nb].rearrange("p t b -> p b t"),
            pt[:, : 4 * nb].rearrange("p (b t) -> p b t", t=4),
            AF.Copy)

    # fuse weights land after the pass-A x chunks are in flight
    WH1 = ld("WH1", [128, KC, C], bf16,
             io["wh1"].ap().rearrange("(kc p) m -> p kc m", p=128))
    WF2X = ld("WF2X", [128, KC, C], bf16,
              io["wf2x"].ap().rearrange("(kc p) m -> p kc m", p=128))
    WF2Y = ld("WF2Y", [L, C], bf16, io["wf2y"].ap())
    stage_load(0)
    stage_load(1)

    # -- A1: products + hw-filter ------------------------------------
    U_IN = ovl.tile([81, 10, BE], f32, tag="big", name="U_IN")
    v.tensor_copy(U_IN[:, 0:4, :], T34[:, :, :])
    for c in range(2):
        s_ = T34[:, c, :]
        t_ = T34[:, 2 + c, :]
        v.tensor_mul(U_IN[:, 4 + c, :], s_, s_)
        v.tensor_mul(U_IN[:, 6 + c, :], t_, t_)
        v.tensor_mul(U_IN[:, 8 + c, :], s_, t_)
    psU = ps_a.tile([81, 10 * BE], f32, tag="pa", name="psU")
    te.matmul(psU[:], SHW[:], U_IN[:, :, :], start=True, stop=True)
    UF = ovl.tile([81, 10, BE], f32, tag="big", name="UF")
    sc.activation(UF[:, :, :], psU[:].rearrange("p (m b) -> p m b", b=BE),
                  AF.Copy)

    stage_f1(0, [0, 1, 2])

    # -- A2: reverse transposes --------------------------------------
    UT = ovl.tile([BE, 10, 81], f32, tag="big", name="UT")
    for m0 in range(0, 10, 6):
        nm = min(6, 10 - m0)
        pt2 = ps_a.tile([BE, 6 * 81], f32, tag="pa", name=f"pt2{m0}")
        for i in range(nm):
            te.transpose(pt2[:, 81 * i : 81 * (i + 1)],
                         UF[:, m0 + i, :], IDF[0:81, 0:81])
        sc.activation(UT[:, m0 : m0 + nm, :],
                      pt2[:, : 81 * nm].rearrange("p (m q) -> p m q", q=81),
                      AF.Copy)
    TT34 = ovl.tile([BL, 4, 81], f32, tag="big", name="TT34")
    pt3 = ps_a.tile([BL, 4 * 81], f32, tag="pa", name="pt3")
    for i in range(4):
        te.transpose(pt3[:, 81 * i : 81 * (i + 1)],
                     T34[:, i, HALO : HALO + BL], IDF[0:81, 0:81])
    sc.activation(TT34[:, :, :],
                  pt3[:].rearrange("p (m q) -> p m q", q=81), AF.Copy)

    stage_f1(0, [3, 4, 5])

    # -- A3: batch filter --------------------------------------------
    UU = ovl.tile([BL, 10, 81], f32, tag="big", name="UU")
    for m0 in range(0, 10, 5):
        pu = ps_a.tile([BL, 5 * 81], f32, tag="pa", name=f"pu{m0}")
        for i in range(5):
            te.matmul(pu[:, 81 * i : 81 * (i + 1)], SB[:], UT[:, m0 + i, :],
                      start=True, stop=True)
        sc.activation(UU[:, m0 : m0 + 5, :],
                      pu[:].rearrange("p (m q) -> p m q", q=81), AF.Copy)

    stage_f1(1, [0, 1, 2])

    # -- A4: ssim arithmetic -----------------------------------------
    SS = ovl.tile([BL, 2, 81], f32, tag="big", name="SS")
    Z = ovl.tile([BL, 2, 81], f32, tag="big", name="Z")
    for c in range(2):
        ux, uy = UU[:, c, :], UU[:, 2 + c, :]
        uxx, uyy, uxy = UU[:, 4 + c, :], UU[:, 6 + c, :], UU[:, 8 + c, :]
        w1 = wA_pool.tile([BL, 81], f32, tag="wa", bufs=6, name=f"w1c{c}")
        w2 = wA_pool.tile([BL, 81], f32, tag="wa", bufs=6, name=f"w2c{c}")
        w3 = wA_pool.tile([BL, 81], f32, tag="wa", bufs=6, name=f"w3c{c}")
        w4 = wA_pool.tile([BL, 81], f32, tag="wa", bufs=6, name=f"w4c{c}")
        w5 = wA_pool.tile([BL, 81], f32, tag="wa", bufs=6, name=f"w5c{c}")
        v.tensor_mul(w1[:], ux, uy)
        v.tensor_mul(w2[:], ux, ux)
        v.tensor_mul(w3[:], uy, uy)
        v.tensor_add(w4[:], w2[:], w3[:])
        v.tensor_scalar(w2[:], w1[:], 2.0, C1S, ALU.mult, ALU.add)
        v.tensor_sub(w3[:], uxy, w1[:])
        v.tensor_scalar(w1[:], w3[:], 2.0 * COV, C2S, ALU.mult, ALU.add)
        v.tensor_scalar(w3[:], w4[:], 1.0, C1S, ALU.mult, ALU.add)
        v.tensor_add(w5[:], uxx, uyy)
        v.tensor_sub(w5[:], w5[:], w4[:])
        v.tensor_scalar(w5[:], w5[:], COV, C2S, ALU.mult, ALU.add)
        v.tensor_mul(w2[:], w2[:], w1[:])
        v.tensor_mul(w3[:], w3[:], w5[:])
        w6 = wA_pool.tile([BL, 81], f32, tag="wa", bufs=6, name=f"w6c{c}")
        sc.activation(w3[:], w3[:], AF.Ln)
        sc.activation(w6[:], w3[:], AF.Exp, scale=-1.0)
        v.tensor_mul(SS[:, c, :], w2[:], w6[:])
        v.tensor_mul(w1[:], SS[:, c, :], TT34[:, c, :])
        v.tensor_add(Z[:, c, :], w1[:], TT34[:, 2 + c, :])

    F22T = ovl.tile([BL, 81], f32, tag="big", name="F22T")
    SSCC = ovl.tile([BL, 81], f32, tag="big", name="SSCC")
    wz = wA_pool.tile([BL, 81], f32, tag="wa", bufs=6, name="wz")
    v.tensor_scalar(wz[:], Z[:, 1, :], BV[0:BL, BV_W01 : BV_W01 + 1],
                    BV[0:BL, BV_BCC : BV_BCC + 1], ALU.mult, ALU.add)
    v.scalar_tensor_tensor(F22T[:], Z[:, 0, :],
                           BV[0:BL, BV_W00 : BV_W00 + 1], wz[:],
                           ALU.mult, ALU.add)
    wz2 = wA_pool.tile([BL, 81], f32, tag="wa", bufs=6, name="wz2")
    v.tensor_scalar(wz2[:], SS[:, 1, :], BV[0:BL, BV_W01 : BV_W01 + 1],
                    BV[0:BL, BV_BCC : BV_BCC + 1], ALU.mult, ALU.add)
    v.scalar_tensor_tensor(SSCC[:], SS[:, 0, :],
                           BV[0:BL, BV_W00 : BV_W00 + 1], wz2[:],
                           ALU.mult, ALU.add)

    stage_f1(1, [3, 4, 5])

    # -- A5: fc1 (pool conv folded on host) --------------------------
    ptr = ps_a.tile([81, BL], f32, tag="pa", name="ptrF22")
    te.transpose(ptr[:], F22T[:], IDF[0:BL, 0:BL])
    F22TT = ovl.tile([81, BL], bf16, tag="big", name="F22TT")
    sc.activation(F22TT[:], ptr[:], AF.Copy)

    H1S = ovl.tile([128, 3, BL], bf16, tag="big", name="H1S")
    nc.gpsimd.memset(H1S[:], 0.0)
    for mo in range(3):
        osz = min(128, 324 - mo * 128)
        pf = ps_a.tile([128, BL], f32, tag="pa", name=f"pf{mo}")
        te.matmul(pf[0:osz, :], WFC1[:, mo * 128 : mo * 128 + osz],
                  F22TT[:], start=True, stop=True)
        sc.activation(H1S[0:osz, mo, :], pf[0:osz, :], AF.Gelu,
                      bias=BV[0:osz, BV_BFC1 + mo : BV_BFC1 + mo + 1])

    stage_f2(0, [0, 1, 2])

    # -- A6: fc2 + leaky + linearize ---------------------------------
    pxw = ps_a.tile([81, BL], f32, tag="pa", name="pxw")
    for k in range(3):
        te.matmul(pxw[:], WFC2[:, k, :], H1S[:, k, :],
                  start=(k == 0), stop=(k == 2))
    XWT = ovl.tile([81, BL], f32, tag="big", name="XWT")
    sc.activation(XWT[:], pxw[:], AF.Lrelu,
                  bias=BV[0:81, BV_BFC2 : BV_BFC2 + 1], alpha=0.01)
    ptw = ps_a.tile([BL, 81], f32, tag="pa", name="ptw")
    te.transpose(ptw[:], XWT[:], IDF[0:81, 0:81])
    XWB = ovl.tile([BL, 81], f32, tag="big", name="XWB")
    sc.activation(XWB[:], ptw[:], AF.Copy)

    # linearize [BL, 81] -> b-major rows staged in DRAM; tiles load slices
    lin_scr = []
    for i, srct in enumerate((F22T, SSCC, XWB)):
        cb = wA_pool.tile([BL, 81], bf16, tag="wcb", name=f"cb{i}")
        v.tensor_copy(cb[:], srct[:, :])
        scr = nc.dram_tensor(f"lin_scr{i}", [BL, 81], bf16, kind="Internal")
        nc.sync.dma_start(out=scr.ap(), in_=cb[:, :])
        lin_scr.append(scr)

    stage_f2(0, [3, 4, 5])

    # six per-kc winograd point-value tiles; these reuse the pass-A slots
    # of the "big" tag (every pass-A tile above is dead before these fill)
    val_ck = [ovl.tile([128, NPT, NTL, BL], f16, tag="big", name=f"vck{k}")
              for k in range(KC)]

    for g in range(N_TILES):
        stage_lrows(g)
        if g >= 1 and g + 1 < N_TILES:
            stage_load(g + 1)
        stage_fold_red(g, "r2")
        if g >= 1 and g + 1 < N_TILES:
            stage_f1(g + 1, [0])
        stage_fold_red(g, "r3")
        stage_cor1_q(g)
        if g >= 1 and g + 1 < N_TILES:
            stage_f1(g + 1, [1])
        stage_fold_red(g, "r1")
        if g >= 1 and g + 1 < N_TILES:
            stage_f1(g + 1, [2])
        stage_cor1_bb(g)
        stage_fold_red(g, "r6")
        if g >= 1 and g + 1 < N_TILES:
            stage_f1(g + 1, [3])
        stage_fold_red(g, "r7")
        if g >= 1 and g + 1 < N_TILES:
            stage_f1(g + 1, [4, 5])
        stage_algebra(g)
        stage_fuse21(g)
        if g + 1 < N_TILES:
            stage_f2(g + 1, range(MO))
        stage_cor2(g)
        stage_cor2_bcast(g)
        stage_h(g)
        for i in range(G):
            stage_w1w2(g, i)
    phase2()

def _split_excess_waits(nc, limit=_SYNC_WAIT_LIMIT):
    """walrus allows only a couple of sem waits per instruction; move any
    excess onto same-engine nops inserted right before the instruction."""
    import bass_rust

    cnt = 0
    for f in nc.m.functions:
        for b in f.blocks:
            insts = b.instructions
            newlist = []
            changed = False
            for inst in insts:
                si = getattr(inst, "sync_info", None)
                waits = list(si.on_wait) if si is not None else []
                if len(waits) > limit:
                    changed = True
                    extra, keep = waits[:-limit], waits[-limit:]
                    for j in range(0, len(extra), limit):
                        nop = mybir.InstNoOp(name=f"waitnop_{cnt}", ins=[],
                                             outs=[])
                        cnt += 1
                        nop.engine = inst.engine
                        nop.sync_info = bass_rust.SyncInfo(
                            on_wait=extra[j : j + limit], on_update=[])
                        nc.register_instruction(nop, overwrite=True)
                        newlist.append(nop)
                    inst.sync_info = bass_rust.SyncInfo(
                        on_wait=keep, on_update=list(si.on_update))
                newlist.append(inst)
            if changed:
                insts[:] = newlist


_PROGRAM_CACHE = {}


def _build_program():
    if "nc" in _PROGRAM_CACHE:
        return _PROGRAM_CACHE["nc"]
    _patch_drain_wait_limit()
    nc = bass.Bass("TRN2", target_bir_lowering=False, debug=False,
                   num_devices=1)
    io = {}
    io["xe"] = nc.dram_tensor("xe", [C, NE], bf16, kind="ExternalInput")
    io["ye"] = nc.dram_tensor("ye", [L, NE], bf16, kind="ExternalInput")
    io["wh1"] = nc.dram_tensor("wh1", [C, C], bf16, kind="ExternalInput")
    io["wf2x"] = nc.dram_tensor("wf2x", [C, C], bf16, kind="ExternalInput")
    io["wf2y"] = nc.dram_tensor("wf2y", [L, C], bf16, kind="ExternalInput")
    io["a3x"] = nc.dram_tensor("a3x", [C, 2], bf16, kind="ExternalInput")
    io["sy4"] = nc.dram_tensor("sy4", [L, 4], bf16, kind="ExternalInput")
    io["wg"] = nc.dram_tensor("wg", [MO, NPT, 128, KC, 128], f16,
                              kind="ExternalInput")
    io["tinr"] = nc.dram_tensor("tinr", [81, NPTL], f16, kind="ExternalInput")
    io["tinva"] = nc.dram_tensor("tinva", [128, 81], f16,
                                 kind="ExternalInput")
    io["tinvb"] = nc.dram_tensor("tinvb", [98, 81], f16,
                                 kind="ExternalInput")
    io["bnbrep"] = nc.dram_tensor("bnbrep", [MO, BL * 128], f16,
                                  kind="ExternalInput")
    io["wfc1"] = nc.dram_tensor("wfc1", [81, 324], bf16, kind="ExternalInput")
    io["wfc2"] = nc.dram_tensor("wfc2", [384, 81], bf16, kind="ExternalInput")
    io["shw"] = nc.dram_tensor("shw", [81, 81], f32, kind="ExternalInput")
    io["sb"] = nc.dram_tensor("sb", [BE, BL], f32, kind="ExternalInput")
    io["bv"] = nc.dram_tensor("bv", [128, BV_NCOLS], f32, kind="ExternalInput")
    io["out"] = nc.dram_tensor("out", [81, BL, O], f32, kind="ExternalOutput")

    from contextlib import ExitStack

    with tile.TileContext(nc) as tc, ExitStack() as ctx:
        _emit(ctx, nc, tc, io)
    _split_excess_waits(nc)
    _PROGRAM_CACHE["nc"] = nc
    return nc


def _reflect_filter_1d(n, win):
    """uniform_filter1d with reflect ('symmetric') padding as an n x n map."""
    r = win // 2
    s = np.zeros((n, n), np.float64)
    for o in range(n):
        for k in range(o - r, o + r + 1):
            i = k
            if i < 0:
                i = -i - 1
            if i > n - 1:
                i = 2 * n - 1 - i
            s[o, i] += 1.0 / win
    return s


def host_prepare(inputs):
    f64 = np.float64
    x = np.asarray(inputs["x"], np.float32)
    y = np.asarray(inputs["y"], np.float32)
    W11 = np.asarray(inputs["w_conv1_1"], f64)
    wf2x = (W11[:, :C2] @ np.asarray(inputs["w_convh2"], f64)).astype(np.float32)
    wf2y = (W11[:, C2:] @ np.asarray(inputs["w_convl1"], f64)).astype(np.float32)
    b_f2 = (W11[:, :C2] @ np.asarray(inputs["b_convh2"], f64)
            + W11[:, C2:] @ np.asarray(inputs["b_convl1"], f64)
            + np.asarray(inputs["b_conv1_1"], f64)).astype(np.float32)
    w12 = np.asarray(inputs["w_conv1_2"], f64)
    a3x = (w12[:, 0:1] @ np.asarray(inputs["w_convh3"], f64)).astype(np.float32)
    a3y = (w12[:, 1:2] @ np.asarray(inputs["w_convl2"], f64)).astype(np.float32)
    b3 = (w12 @ np.concatenate([np.asarray(inputs["b_convh3"], f64),
                                np.asarray(inputs["b_convl2"], f64)])
          + np.asarray(inputs["b_conv1_2"], f64)).astype(np.float32)
    bias4 = np.concatenate([b3, np.asarray(inputs["b_convl3"], np.float32)])

    sy4 = np.concatenate(
        [a3y.T, np.asarray(inputs["w_convl3"], np.float32).T], axis=1)

    s1 = _reflect_filter_1d(HH, WIN)
    shw = np.kron(s1, s1).T.astype(np.float32)  # lhsT [in_px, out_px]
    sb_m = np.zeros((BE, BL), np.float32)
    for o in range(BL):
        sb_m[o : o + WIN, o] = 1.0 / WIN

    w_pool = np.asarray(inputs["w_pool"], f64)  # (2, 1, 3, 3)
    mconv = np.zeros((2, 81, 81), f64)          # [c, out_px, in_px]
    for c in range(2):
        for oh in range(HH):
            for ow in range(WW):
                for dh in range(3):
                    for dw in range(3):
                        ih, iw = oh + dh - 1, ow + dw - 1
                        if 0 <= ih < HH and 0 <= iw < WW:
                            mconv[c, oh * WW + ow, ih * WW + iw] = \
                                w_pool[c, 0, dh, dw]

    bfd = ml_dtypes.bfloat16
    W1 = np.asarray(inputs["w_fc1"], f64)
    bp = np.asarray(inputs["b_pool"], f64)
    wf = (W1[:, 0:81] + W1[:, 243:324]
          + W1[:, 81:162] @ mconv[0] + W1[:, 162:243] @ mconv[1])
    wfc1 = np.ascontiguousarray(wf.T).astype(bfd)       # lhsT [81, 324]
    bfc1 = (np.asarray(inputs["b_fc1"], f64)
            + bp[0] * W1[:, 81:162].sum(axis=1)
            + bp[1] * W1[:, 162:243].sum(axis=1)).astype(np.float32)
    wfc2 = np.zeros((384, 81), bfd)
    wfc2[:324] = np.asarray(inputs["w_fc2"], np.float32).T.astype(bfd)

    bn_scale = (np.asarray(inputs["bn_gamma"], f64)
                / np.sqrt(np.asarray(inputs["bn_var"], f64) + 1e-5))
    bn_bias = (np.asarray(inputs["bn_beta"], f64)
               - np.asarray(inputs["bn_mean"], f64) * bn_scale)

    bv = np.zeros((128, BV_NCOLS), np.float32)
    b_h1 = np.asarray(inputs["b_convh1"], np.float32)
    for m in range(MO):
        bv[:, BV_BH1 + m] = b_h1[m * 128 : (m + 1) * 128]
        bv[:, BV_BF2 + m] = b_f2[m * 128 : (m + 1) * 128]
        bv[:, BV_BNS + m] = bn_scale[m * 128 : (m + 1) * 128]
        bv[:, BV_BNB + m] = bn_bias[m * 128 : (m + 1) * 128]
    bv[0:2, BV_B4] = bias4[0:2]
    bv[0:2, BV_B4Y] = bias4[2:4]
    for mo in range(3):
        osz = min(128, 324 - mo * 128)
        bv[0:osz, BV_BFC1 + mo] = bfc1[mo * 128 : mo * 128 + osz]
    bv[0:81, BV_BFC2] = np.asarray(inputs["b_fc2"], np.float32)
    bv[:, BV_W00] = np.float32(inputs["w_cc1"][0, 0])
    bv[:, BV_W01] = np.float32(inputs["w_cc1"][0, 1])
    bv[:, BV_BCC] = np.float32(inputs["b_cc1"][0])
    bv[:, BV_BP0] = np.float32(inputs["b_pool"][0])
    bv[:, BV_BP1] = np.float32(inputs["b_pool"][1])

    # winograd transforms + transformed conv weights (bn scale folded in)
    f16n = np.float16
    Gm, Tin, Tinv = _wino_transforms()
    tinr = Tin.T.astype(f16n)                      # [81, 225] rhs
    tinva = Tinv[:, 0:128].T.astype(f16n)          # [128, 81] lhsT chunk A
    tinvb = np.ones((98, 81), np.float64)
    tinvb[0:97] = Tinv[:, 128:NPTL].T              # row 97 stays 1.0 (bias)
    tinvb = tinvb.astype(f16n)
    wb_s = (np.asarray(inputs["w_bconv"], f64)
            * bn_scale[:, None, None, None])       # fold BN scale into conv
    Wg = np.einsum("pi,ocij,qj->pqoc", Gm, wb_s, Gm).reshape(NPT, O, C)
    wg = (Wg.reshape(NPT, MO, 128, KC, 128)
          .transpose(1, 0, 4, 3, 2)).astype(f16n)  # [mo, pt, cp, kc, ocol]
    bnbrep = np.tile(bn_bias.reshape(MO, 1, 128),
                     (1, BL, 1)).reshape(MO, BL * 128).astype(f16n)

    bf = ml_dtypes.bfloat16
    common = {
        "wh1": np.asarray(inputs["w_convh1"], np.float32).T.astype(bf),
        "wf2x": wf2x.T.astype(bf),
        "wf2y": wf2y.T.astype(bf),
        "a3x": a3x.T.astype(bf),
        "sy4": sy4.astype(bf),
        "wg": wg, "tinr": tinr, "tinva": tinva, "tinvb": tinvb,
        "bnbrep": bnbrep,
        "wfc1": wfc1, "wfc2": wfc2,
        "shw": shw, "sb": sb_m, "bv": bv,
    }
    common = {k: np.ascontiguousarray(v) for k, v in common.items()}

    xp = np.pad(x, ((HALO, HALO), (0, 0), (0, 0), (0, 0)), mode="symmetric")
    yp = np.pad(y, ((HALO, HALO), (0, 0), (0, 0), (0, 0)), mode="symmetric")
    in_maps = []
    for m in range(M_CORES):
        xe = np.ascontiguousarray(
            xp[m * BL : m * BL + BE].transpose(1, 0, 2, 3).reshape(C, NE)
        ).astype(bf)
        ye = np.ascontiguousarray(
            yp[m * BL : m * BL + BE].transpose(1, 0, 2, 3).reshape(L, NE)
        ).astype(bf)
        in_maps.append({"xe": xe, "ye": ye, **common})
    return in_maps


def kernel(**inputs):
    nc = _build_program()
    in_maps = host_prepare(inputs)
    trace = os.environ.get("KERNEL_TRACE", "0") == "1"
    kw = {}
    if trace:
        kw = dict(trace=True, trace_cores=[0])
    res = run_bass_kernel_spmd(nc, in_maps, core_ids=list(range(M_CORES)), **kw)
    if trace:
        kernel.last_results = res
        if res.exec_time_ns is not None:
            print(f"HW exec time: {res.exec_time_ns} ns")
    out = np.empty((B, O, HH, WW), np.float32)
    for m in range(M_CORES):
        o = res.results[m]["out"]          # [81, BL, O] pixel-major
        out[m * BL : (m + 1) * BL] = (
            o.reshape(HH, WW, BL, O).transpose(2, 3, 0, 1))
    return out



# revision 45
# speedup vs baseline: 1.7444x; 1.4486x over previous
"""Trainium2 Bass kernel for nn_FAFMoudle (dense_cnn).

Data-parallel across 8 NeuronCores: 32 images per core plus a 3-image halo
on each side for the SSIM uniform filter (which smooths across the batch
axis).  The halo is materialized on the host by symmetrically padding the
global batch, so every core runs an identical program on its own shard.

Device-side plan (per core, all 1x1 convs folded on host into single
matmuls, channel-major layout [C, b*81]):
  pass A: fuse_3/fuse_4 (2ch maps) over the 38 ext images -> SSIM via
          small filter-matrix matmuls (hw-filter 81x81, batch-filter 38x32)
          with PE transposes between; fuse2_2 / cc1(ssim) / xweight
          (fc1+gelu+fc2+leakyrelu) -> linearized per-pixel scalar rows.
  pass B: per 6-image tile: fuse_1/fuse_2 (bf16 matmuls), cosine sims via
          pointwise products + ones-vector PE reductions, fuse2_1/fuse3_1
          chain, xout written into a zero-padded per-image buffer, then the
          3x3 conv as 9*6 accumulating matmuls per output chunk, fused
          BN+leaky-relu on evacuation.
"""

import os
import sys

for _p in (
    "/opt/trn_rl_repo",
    "/root/.axon_site",
    "/root/.axon_site/_ro/trn_rl_repo",
    "/root/.axon_site/_ro/pypackages",
):
    if os.path.isdir(_p) and _p not in sys.path:
        sys.path.insert(0, _p)

import math

import ml_dtypes
import numpy as np

import concourse.bass as bass
import concourse.tile as tile
from concourse import mybir
from concourse.bass_utils import run_bass_kernel_spmd
from concourse.masks import make_identity

dt = mybir.dt
AF = mybir.ActivationFunctionType
ALU = mybir.AluOpType

# ----------------------------------------------------------------------------
# shapes
B, C, L, O, HH, WW = 256, 768, 64, 768, 9, 9
C2, C3 = 2 * C // 3, C // 3
M_CORES = 8
BL = B // M_CORES          # 32 images per core
HALO = 3
BE = BL + 2 * HALO         # 38 ext images
PX = HH * WW               # 81
NV = BL * PX               # 2592 valid pixels
NE = BE * PX               # 3078 ext pixels
KC = C // 128              # 6 contraction chunks
MO = O // 128              # 6 output chunks
G = 6                      # images per pass-B tile
TW = G * PX                # 486
N_TILES = (BL + G - 1) // G
WIN = 7
COV = (WIN ** 3) / (WIN ** 3 - 1.0)
C1S, C2S = 0.01 ** 2, 0.03 ** 2
SQRT_C = math.sqrt(C)
# padded per-image layout for the 3x3 conv input: 11 rows x 12 cols,
# interior at rows 1..9, cols 2..10 (keeps every 9-wide run 4B aligned)
IMR, IMC = 11, 12
IMS = IMR * IMC            # 132

bf16 = dt.bfloat16
f32 = dt.float32

# BV (bias/const matrix) column map
BV_BH1 = 0          # 6 cols
BV_BF2 = 6          # 6 cols
BV_B4 = 12          # 1 col (rows 0:2, f3 bias)
BV_BFC1 = 13        # 3 cols
BV_BFC2 = 16        # 1 col (rows 0:81)
BV_BNS = 17         # 6 cols
BV_BNB = 23         # 6 cols
BV_W00 = 29
BV_W01 = 30
BV_BCC = 31
BV_BP0 = 32
BV_BP1 = 33
BV_B4Y = 34         # f4 bias (rows 0:2)
BV_NCOLS = 35

_SYNC_WAIT_LIMIT = 1


def _patch_drain_wait_limit():
    """walrus in this container only allows 2 sem waits per TPB_CTRL
    instruction; split the tile-exit drain's waits across extra nops."""
    import bass_rust
    from concourse.tile import ScopedClock, TileContext

    if getattr(TileContext, "_drain_waits_patched", False):
        return

    def _drain_and_barrier(self, tick_clock, wait_clock):
        drain_inst = self.nc.sync.drain()
        wait_clock.add_sem_waits(
            drain_inst.ins, ScopedClock({None: tick_clock.global_clock})
        )
        si = drain_inst.ins.sync_info
        waits = list(si.on_wait)
        if len(waits) > _SYNC_WAIT_LIMIT:
            drain_inst.ins.sync_info = bass_rust.SyncInfo(
                on_wait=waits[:_SYNC_WAIT_LIMIT], on_update=list(si.on_update)
            )
            for i in range(_SYNC_WAIT_LIMIT, len(waits), _SYNC_WAIT_LIMIT):
                n = self.nc.sync.nop()
                n.ins.sync_info = bass_rust.SyncInfo(
                    on_wait=waits[i : i + _SYNC_WAIT_LIMIT], on_update=[]
                )
        self.nc.all_engine_barrier()
        popped = self.nc._tile_sem_poison_stack.pop()
        assert popped is self._sem_poison
        self.nc.clear_and_free_semaphores(list(self.sems.allocated().values()))
        self.nc.all_engine_barrier()

    TileContext._drain_and_barrier = _drain_and_barrier
    TileContext._drain_waits_patched = True


def _emit(ctx, nc, tc, io):
    v = nc.vector
    sc = nc.scalar
    te = nc.tensor

    cp = ctx.enter_context(tc.tile_pool(name="const", bufs=1))
    pp = ctx.enter_context(tc.tile_pool(name="persist", bufs=1))
    xt_pool = ctx.enter_context(tc.tile_pool(name="xt", bufs=2))
    f_pool = ctx.enter_context(tc.tile_pool(name="fs", bufs=2))
    prod_pool = ctx.enter_context(tc.tile_pool(name="prod", bufs=6))
    bcs_pool = ctx.enter_context(tc.tile_pool(name="bcs", bufs=2))
    sc_pool = ctx.enter_context(tc.tile_pool(name="sct", bufs=3))
    xp_pool = ctx.enter_context(tc.tile_pool(name="xp", bufs=2))
    out_pool = ctx.enter_context(tc.tile_pool(name="outp", bufs=2))
    wA_pool = ctx.enter_context(tc.tile_pool(name="wA", bufs=1))

    ps_a = ctx.enter_context(tc.tile_pool(name="psA", bufs=3, space="PSUM"))
    ps_red = ctx.enter_context(tc.tile_pool(name="psRed", bufs=1, space="PSUM"))

    # ---- constants / weights into SBUF --------------------------------
    def ld(name, shape, dtype, ap):
        t = cp.tile(shape, dtype, name=name)
        nc.sync.dma_start(out=t[:], in_=ap)
        return t

    A3X = ld("A3X", [128, KC, 2], bf16,
             io["a3x"].ap().rearrange("(kc p) m -> p kc m", p=128))
    SY4 = ld("SY4", [L, 4], bf16, io["sy4"].ap())
    BV = ld("BV", [128, BV_NCOLS], f32, io["bv"].ap())
    ye_ap = io["ye"].ap()
    xe_re0 = io["xe"].ap().rearrange("(kc p) n -> p kc n", p=128)

    # first pass-A chunk DMAs go out before the bulky consts so the PE can
    # start as soon as possible
    preA = {}

    def loadA(c0, w):
        xa = xt_pool.tile([128, KC, TW], bf16, tag="xt", name=f"xa{c0}")
        nc.sync.dma_start(out=xa[:, :, :w], in_=xe_re0[:, :, c0 : c0 + w])
        ya = xt_pool.tile([L, TW], bf16, tag="yt", name=f"ya{c0}")
        nc.sync.dma_start(out=ya[:, :w], in_=ye_ap[:, c0 : c0 + w])
        return xa, ya

    preA[0] = loadA(0, min(TW, NE))

    WB = cp.tile([128, 9, KC, O], bf16, name="WB")   # DMA'd after pass A
    WFC1 = ld("WFC1", [81, 324], bf16, io["wfc1"].ap())
    WFC2 = ld("WFC2", [128, 3, 81], bf16,
              io["wfc2"].ap().rearrange("(kc p) m -> p kc m", p=128))
    SHW = ld("SHW", [81, 81], f32, io["shw"].ap())
    SB = ld("SB", [BE, BL], f32, io["sb"].ap())

    IDF = cp.tile([128, 128], f32, name="IDF")
    make_identity(nc, IDF[:])
    ONESC = cp.tile([128, 1], bf16, name="ONESC")
    nc.gpsimd.memset(ONESC[:], 1.0)
    ONESR = cp.tile([1, 128], bf16, name="ONESR")
    nc.gpsimd.memset(ONESR[:], 1.0)
    EPSR = cp.tile([1, 1], f32, name="EPSR")
    nc.gpsimd.memset(EPSR[:], 1e-16)

    xe_re = io["xe"].ap().rearrange("(kc p) n -> p kc n", p=128)
    out_re = io["out"].ap().rearrange("(mo p) n -> p mo n", p=128)

    st = {}

    def tdims(g):
        gi = min(G, BL - g * G)
        return gi, gi * PX, g * TW, HALO * PX + g * TW

    def stage_load(g):
        gi, w, c0, ce = tdims(g)
        xt = xt_pool.tile([128, KC, TW], bf16, tag="xt", name=f"xt{g}")
        nc.sync.dma_start(out=xt[:, :, :w], in_=xe_re[:, :, ce : ce + w])
        yt = xt_pool.tile([L, TW], bf16, tag="yt", name=f"yt{g}")
        nc.sync.dma_start(out=yt[:, :w], in_=ye_ap[:, ce : ce + w])
        st[g] = {"xt": xt, "yt": yt}

    def stage_f1(g, ms):
        gi, w, c0, ce = tdims(g)
        s = st[g]
        if "F1S" not in s:
            s["F1S"] = f_pool.tile([128, KC, TW], bf16, tag="f1s",
                                   name=f"f1s{g}")
        F1S = s["F1S"]
        for m in ms:
            p1 = ps_a.tile([128, TW], f32, tag="pa", name=f"p1_{g}_{m}")
            for k in range(KC):
                te.matmul(p1[:, :w], WH1[:, k, m * 128 : (m + 1) * 128],
                          s["xt"][:, k, :w], start=(k == 0),
                          stop=(k == KC - 1))
            sc.activation(F1S[:, m, :w], p1[:, :w], AF.Identity,
                          bias=BV[:, BV_BH1 + m : BV_BH1 + m + 1])

    def stage_f2(g, ms):
        gi, w, c0, ce = tdims(g)
        s = st[g]
        if "F2S" not in s:
            s["F2S"] = f_pool.tile([128, KC, TW], bf16, tag="f2s", bufs=2,
                                   name=f"f2s{g}")
        F2S = s["F2S"]
        for m in ms:
            p2 = ps_a.tile([128, TW], f32, tag="pa", name=f"p2_{g}_{m}")
            te.matmul(p2[:, :w], WF2Y[:, m * 128 : (m + 1) * 128],
                      s["yt"][:, :w], start=True, stop=False)
            for k in range(KC):
                te.matmul(p2[:, :w], WF2X[:, k, m * 128 : (m + 1) * 128],
                          s["xt"][:, k, :w], start=False, stop=(k == KC - 1))
            sc.activation(F2S[:, m, :w], p2[:, :w], AF.Identity,
                          bias=BV[:, BV_BF2 + m : BV_BF2 + m + 1])

    def stage_fold_red(g, which):
        # 6-fold the channel-chunk terms on DVE, then one M=1 matmul into
        # a packed psum row (rows 32-aligned so groups stay independent)
        gi, w, c0, ce = tdims(g)
        s = st[g]
        F1S, F2S = s["F1S"], s["F2S"]
        if "rr" not in s:
            s["rr"] = ps_red.tile([1, 5 * 512], f32, tag="red", name=f"rr_{g}")
        spec = {
            "r1": (0, F1S, F2S),
            "r2": (1, F1S, F1S),
            "r3": (2, F2S, F2S),
            "r6": (3, F1S, None),
            "r7": (4, F2S, None),
        }
        slot, a, b = spec[which]
        rt = s["rr"]
        acc = prod_pool.tile([128, TW], bf16, tag="pp", name=f"ac{which}{g}")
        if b is None:
            v.tensor_add(acc[:, :w], a[:, 0, :w], a[:, 1, :w])
            for m in range(2, MO):
                v.tensor_add(acc[:, :w], acc[:, :w], a[:, m, :w])
        else:
            v.tensor_mul(acc[:, :w], a[:, 0, :w], b[:, 0, :w])
            for m in range(1, MO):
                tmp = prod_pool.tile([128, TW], bf16, tag="pp",
                                     name=f"tp{which}{g}_{m}")
                v.tensor_mul(tmp[:, :w], a[:, m, :w], b[:, m, :w])
                v.tensor_add(acc[:, :w], acc[:, :w], tmp[:, :w])
        te.matmul(rt[0:1, 512 * slot : 512 * slot + w], ONESC[:],
                  acc[:, :w], start=True, stop=True)

    def stage_cor1_q(g):
        # issued right after the r2/r3 folds: the 1/sqrt(r2*r3) chain runs
        # on scalar while the r1 fold is still going on vector
        gi, w, c0, ce = tdims(g)
        s = st[g]
        rr = s["rr"]
        r2 = rr[0:1, 512 : 512 + TW]
        r3 = rr[0:1, 1024 : 1024 + TW]
        q1 = sc_pool.tile([1, TW], f32, tag="scf", bufs=5, name=f"q1_{g}")
        q3 = sc_pool.tile([1, TW], f32, tag="scf", bufs=5, name=f"q3_{g}")
        qs = sc_pool.tile([1, TW], f32, tag="scf", bufs=5, name=f"qs_{g}")
        sc.activation(q3[:, :w], r3[:, :w], AF.Copy)
        v.tensor_mul(qs[:, :w], r2[:, :w], q3[:, :w])
        # 1/sqrt(x) as exp(-0.5*ln(x)) -- keeps the whole chain on the scalar
        # engine instead of DVE's ~3.8us iterative reciprocal
        sc.activation(qs[:, :w], qs[:, :w], AF.Ln, bias=EPSR[0:1, 0:1])
        sc.activation(q1[:, :w], qs[:, :w], AF.Exp, scale=-0.5)
        s["q1"] = q1

    def stage_cor1_bb(g):
        gi, w, c0, ce = tdims(g)
        s = st[g]
        rr = s["rr"]
        r1 = rr[0:1, 0:TW]
        q1 = s["q1"]
        beta = sc_pool.tile([1, TW], bf16, tag="scb", name=f"beta{g}")
        q2 = sc_pool.tile([1, TW], f32, tag="scf", bufs=5, name=f"q2_{g}")
        v.scalar_tensor_tensor(q2[:, :w], r1[:, :w], -0.5, q1[:, :w],
                               ALU.mult, ALU.mult)
        v.tensor_scalar_add(beta[:, :w], q2[:, :w], 0.5)
        s["beta"] = beta
        bb = ps_a.tile([128, TW], f32, tag="pa", name=f"bb{g}")
        te.matmul(bb[:, :w], ONESR[:], beta[:, :w], start=True, stop=True)
        bbs = bcs_pool.tile([128, TW], bf16, tag="bcs", name=f"bbs{g}")
        sc.activation(bbs[:, :w], bb[:, :w], AF.Copy)
        s["bbs"] = bbs

    def stage_algebra(g):
        gi, w, c0, ce = tdims(g)
        s = st[g]
        rr = s["rr"]
        r1 = rr[0:1, 0:TW]
        r2 = rr[0:1, 512 : 512 + TW]
        r3 = rr[0:1, 1024 : 1024 + TW]
        r6 = rr[0:1, 1536 : 1536 + TW]
        r7 = rr[0:1, 2048 : 2048 + TW]
        beta = s["beta"]
        # r4 = r6 + beta*r7   (fuse2_1 channel-sum, no extra reduction)
        r4s = sc_pool.tile([1, TW], f32, tag="scf", bufs=5, name=f"r4s_{g}")
        v.tensor_mul(r4s[:, :w], beta[:, :w], r7[:, :w])
        v.tensor_add(r4s[:, :w], r4s[:, :w], r6[:, :w])
        s["r4s"] = r4s
        # r5 = r2 + 2*beta*r1 + beta^2*r3
        t1 = sc_pool.tile([1, TW], f32, tag="scf", bufs=5, name=f"t1_{g}")
        t2 = sc_pool.tile([1, TW], f32, tag="scf", bufs=5, name=f"t2_{g}")
        v.tensor_mul(t1[:, :w], beta[:, :w], r1[:, :w])
        v.tensor_mul(t2[:, :w], beta[:, :w], r3[:, :w])
        v.tensor_mul(t2[:, :w], beta[:, :w], t2[:, :w])
        v.scalar_tensor_tensor(t1[:, :w], t1[:, :w], 2.0, t2[:, :w],
                               ALU.mult, ALU.add)
        v.tensor_add(t1[:, :w], t1[:, :w], r2[:, :w])
        s["r5s"] = t1

    def stage_fuse21(g):
        gi, w, c0, ce = tdims(g)
        s = st[g]
        F1S, F2S, bbs = s["F1S"], s["F2S"], s["bbs"]
        for m in range(MO):
            td = prod_pool.tile([128, TW], bf16, tag="pp", name=f"td{g}_{m}")
            v.tensor_mul(td[:, :w], bbs[:, :w], F2S[:, m, :w])
            # fuse2_1 overwrites F1S in place
            v.tensor_add(F1S[:, m, :w], td[:, :w], F1S[:, m, :w])

    def stage_lrows(g):
        gi, w, c0, ce = tdims(g)
        s = st[g]
        for nm_, idx in (("f22l", 0), ("sccl", 1), ("xwl", 2)):
            t_ = sc_pool.tile([1, TW], bf16, tag="l" + nm_, bufs=2,
                              name=f"{nm_}{g}")
            nc.sync.dma_start(
                out=t_[0:1, :w],
                in_=lin_scr[idx].ap().rearrange(
                    "(one b) q -> one (b q)", one=1)[:, c0 : c0 + w])
            s[nm_] = t_

    def stage_cor2(g):
        gi, w, c0, ce = tdims(g)
        s = st[g]
        r4s, r5s = s["r4s"], s["r5s"]
        f22l = s["f22l"]
        nmr = sc_pool.tile([1, TW], f32, tag="scf", bufs=5, name=f"nm{g}")
        v.tensor_mul(nmr[:, :w], f22l[:, :w], r4s[:, :w])
        # 1/(sqrt(r5)*|f22l|*sqrt(C)) = exp(-0.5*ln(r5*f22l^2*C))
        s5 = sc_pool.tile([1, TW], f32, tag="scf", bufs=5, name=f"s5_{g}")
        af_ = sc_pool.tile([1, TW], f32, tag="scf", bufs=5, name=f"af{g}")
        v.tensor_mul(af_[:, :w], f22l[:, :w], f22l[:, :w])
        v.tensor_mul(s5[:, :w], r5s[:, :w], af_[:, :w])
        sc.activation(s5[:, :w], s5[:, :w], AF.Ln, scale=float(C), bias=EPSR[0:1, 0:1])
        s5i = sc_pool.tile([1, TW], f32, tag="scf", bufs=5, name=f"s5i_{g}")
        sc.activation(s5i[:, :w], s5[:, :w], AF.Exp, scale=-0.5)
        v.tensor_mul(nmr[:, :w], nmr[:, :w], s5i[:, :w])    # cor2
        v.tensor_sub(nmr[:, :w], nmr[:, :w], s["sccl"][:, :w])
        v.tensor_scalar(nmr[:, :w], nmr[:, :w], -0.5, 0.5, ALU.mult, ALU.add)
        delta = sc_pool.tile([1, TW], bf16, tag="scb", name=f"dl{g}")
        v.tensor_mul(delta[:, :w], nmr[:, :w], f22l[:, :w])
        s["delta"] = delta
        xw1 = sc_pool.tile([1, TW], bf16, tag="scb", name=f"xw1_{g}")
        v.tensor_scalar_add(xw1[:, :w], s["xwl"][:, :w], 1.0)
        s["xw1"] = xw1

    def stage_cor2_bcast(g):
        gi, w, c0, ce = tdims(g)
        s = st[g]
        bd = ps_a.tile([128, TW], f32, tag="pa", name=f"bd{g}")
        te.matmul(bd[:, :w], ONESR[:], s["delta"][:, :w], start=True,
                  stop=True)
        dbs = bcs_pool.tile([128, TW], bf16, tag="bcs", name=f"dbs{g}")
        sc.activation(dbs[:, :w], bd[:, :w], AF.Copy)
        s["dbs"] = dbs
        bw = ps_a.tile([128, TW], f32, tag="pa", name=f"bw{g}")
        te.matmul(bw[:, :w], ONESR[:], s["xw1"][:, :w], start=True, stop=True)
        wbs = bcs_pool.tile([128, TW], bf16, tag="bcs", name=f"wbs{g}")
        sc.activation(wbs[:, :w], bw[:, :w], AF.Copy)
        s["wbs"] = wbs

    def stage_xout(g):
        gi, w, c0, ce = tdims(g)
        s = st[g]
        F1S, dbs, wbs = s["F1S"], s["dbs"], s["wbs"]
        XP = xp_pool.tile([128, KC, G, IMS], bf16, tag="xp", name=f"xp{g}")
        nc.gpsimd.memset(XP[:], 0.0)
        for m in range(MO):
            t3 = prod_pool.tile([128, TW], bf16, tag="pp", name=f"t3{g}_{m}")
            v.tensor_add(t3[:, :w], F1S[:, m, :w], dbs[:, :w])
            xpv = XP[:, m, :, :].rearrange("p im (r c) -> p im r c", c=IMC)
            v.tensor_mul(
                xpv[:, :gi, 1:10, 2:11],
                t3[:, :w].rearrange("p (im r c) -> p im r c", r=HH, c=WW),
                wbs[:, :w].rearrange("p (im r c) -> p im r c", r=HH, c=WW))
        s["XP"] = XP

    def stage_bconv(g, ms):
        gi, w, c0, ce = tdims(g)
        s = st[g]
        XP = s["XP"]
        for m in ms:
            pb2 = ps_a.tile([128, TW], f32, tag="pa", name=f"pbc{g}_{m}")
            for d in range(9):
                di, dj = d // 3, d % 3
                for k in range(KC):
                    mv = XP[:, k, :, :].rearrange(
                        "p im (r c) -> p im r c", c=IMC
                    )[:, :gi, di : di + 9, dj + 1 : dj + 10]
                    te.matmul(pb2[:, :w], WB[:, d, k, m * 128 : (m + 1) * 128],
                              mv, start=(d == 0 and k == 0),
                              stop=(d == 8 and k == KC - 1))
            ot = out_pool.tile([128, TW], f32, tag="ot", name=f"ot{g}_{m}")
            sc.activation(ot[:, :w], pb2[:, :w], AF.Lrelu,
                          scale=BV[:, BV_BNS + m : BV_BNS + m + 1],
                          bias=BV[:, BV_BNB + m : BV_BNB + m + 1],
                          alpha=0.01)
            nc.sync.dma_start(out=out_re[:, m, c0 : c0 + w], in_=ot[:, :w])


    # =========================== pass A ================================
    # fuse_3 / fuse_4 over ext pixels, transposed per image into
    # T34 [81, (t, b)] with t in {f3c0, f3c1, f4c0, f4c1}
    T34 = pp.tile([81, 4, BE], f32, name="T34")
    chunksA = [(c0, min(TW, NE - c0)) for c0 in range(0, NE, TW)]
    for c0, w in chunksA:
        nb = w // PX
        b0 = c0 // PX
        if c0 in preA:
            xa, ya = preA.pop(c0)
        else:
            xa, ya = loadA(c0, w)
        f3p = ps_a.tile([2, TW], f32, tag="pa", name=f"f3p{c0}")
        f4p = ps_a.tile([2, TW], f32, tag="pa", name=f"f4p{c0}")
        te.matmul(f4p[:, :w], SY4[:, 2:4], ya[:, :w], start=True, stop=True)
        te.matmul(f3p[:, :w], SY4[:, 0:2], ya[:, :w], start=True, stop=False)
        for k in range(KC):
            te.matmul(f3p[:, :w], A3X[:, k, :], xa[:, k, :w],
                      start=False, stop=(k == KC - 1))
        f3s = xt_pool.tile([2, TW], f32, tag="f3s", bufs=1, name=f"f3s{c0}")
        f4s = xt_pool.tile([2, TW], f32, tag="f4s", bufs=1, name=f"f4s{c0}")
        sc.activation(f3s[:, :w], f3p[:, :w], AF.Identity,
                      bias=BV[0:2, BV_B4 : BV_B4 + 1])
        sc.activation(f4s[:, :w], f4p[:, :w], AF.Identity,
                      bias=BV[0:2, BV_B4Y : BV_B4Y + 1])
        pt = ps_a.tile([81, 4 * G], f32, tag="pa", name=f"pt{c0}")
        for i in range(nb):
            te.transpose(pt[:, 4 * i : 4 * i + 2],
                         f3s[:, i * 81 : (i + 1) * 81], IDF[0:2, 0:2])
            te.transpose(pt[:, 4 * i + 2 : 4 * i + 4],
                         f4s[:, i * 81 : (i + 1) * 81], IDF[0:2, 0:2])
        sc.activation(
            T34[:, :, b0 : b0 + nb].rearrange("p t b -> p b t"),
            pt[:, : 4 * nb].rearrange("p (b t) -> p b t", t=4),
            AF.Copy)

    # fuse weights land after the pass-A x chunks are in flight
    WH1 = ld("WH1", [128, KC, C], bf16,
             io["wh1"].ap().rearrange("(kc p) m -> p kc m", p=128))
    WF2X = ld("WF2X", [128, KC, C], bf16,
              io["wf2x"].ap().rearrange("(kc p) m -> p kc m", p=128))
    WF2Y = ld("WF2Y", [L, C], bf16, io["wf2y"].ap())
    stage_load(0)
    stage_load(1)

    # -- A1: products + hw-filter ------------------------------------
    U_IN = pp.tile([81, 10, BE], f32, name="U_IN")
    v.tensor_copy(U_IN[:, 0:4, :], T34[:, :, :])
    for c in range(2):
        s_ = T34[:, c, :]
        t_ = T34[:, 2 + c, :]
        v.tensor_mul(U_IN[:, 4 + c, :], s_, s_)
        v.tensor_mul(U_IN[:, 6 + c, :], t_, t_)
        v.tensor_mul(U_IN[:, 8 + c, :], s_, t_)
    psU = ps_a.tile([81, 10 * BE], f32, tag="pa", name="psU")
    te.matmul(psU[:], SHW[:], U_IN[:, :, :], start=True, stop=True)
    UF = pp.tile([81, 10, BE], f32, name="UF")
    sc.activation(UF[:, :, :], psU[:].rearrange("p (m b) -> p m b", b=BE),
                  AF.Copy)

    stage_f1(0, [0, 1, 2])

    # -- A2: reverse transposes --------------------------------------
    UT = pp.tile([BE, 10, 81], f32, name="UT")
    for m0 in range(0, 10, 6):
        nm = min(6, 10 - m0)
        pt2 = ps_a.tile([BE, 6 * 81], f32, tag="pa", name=f"pt2{m0}")
        for i in range(nm):
            te.transpose(pt2[:, 81 * i : 81 * (i + 1)],
                         UF[:, m0 + i, :], IDF[0:81, 0:81])
        sc.activation(UT[:, m0 : m0 + nm, :],
                      pt2[:, : 81 * nm].rearrange("p (m q) -> p m q", q=81),
                      AF.Copy)
    TT34 = pp.tile([BL, 4, 81], f32, name="TT34")
    pt3 = ps_a.tile([BL, 4 * 81], f32, tag="pa", name="pt3")
    for i in range(4):
        te.transpose(pt3[:, 81 * i : 81 * (i + 1)],
                     T34[:, i, HALO : HALO + BL], IDF[0:81, 0:81])
    sc.activation(TT34[:, :, :],
                  pt3[:].rearrange("p (m q) -> p m q", q=81), AF.Copy)

    stage_f1(0, [3, 4, 5])

    # -- A3: batch filter --------------------------------------------
    UU = pp.tile([BL, 10, 81], f32, name="UU")
    for m0 in range(0, 10, 5):
        pu = ps_a.tile([BL, 5 * 81], f32, tag="pa", name=f"pu{m0}")
        for i in range(5):
            te.matmul(pu[:, 81 * i : 81 * (i + 1)], SB[:], UT[:, m0 + i, :],
                      start=True, stop=True)
        sc.activation(UU[:, m0 : m0 + 5, :],
                      pu[:].rearrange("p (m q) -> p m q", q=81), AF.Copy)

    stage_f1(1, [0, 1, 2])

    # -- A4: ssim arithmetic -----------------------------------------
    SS = pp.tile([BL, 2, 81], f32, name="SS")
    Z = pp.tile([BL, 2, 81], f32, name="Z")
    for c in range(2):
        ux, uy = UU[:, c, :], UU[:, 2 + c, :]
        uxx, uyy, uxy = UU[:, 4 + c, :], UU[:, 6 + c, :], UU[:, 8 + c, :]
        w1 = wA_pool.tile([BL, 81], f32, tag="wa", bufs=6, name=f"w1c{c}")
        w2 = wA_pool.tile([BL, 81], f32, tag="wa", bufs=6, name=f"w2c{c}")
        w3 = wA_pool.tile([BL, 81], f32, tag="wa", bufs=6, name=f"w3c{c}")
        w4 = wA_pool.tile([BL, 81], f32, tag="wa", bufs=6, name=f"w4c{c}")
        w5 = wA_pool.tile([BL, 81], f32, tag="wa", bufs=6, name=f"w5c{c}")
        v.tensor_mul(w1[:], ux, uy)
        v.tensor_mul(w2[:], ux, ux)
        v.tensor_mul(w3[:], uy, uy)
        v.tensor_add(w4[:], w2[:], w3[:])
        v.tensor_scalar(w2[:], w1[:], 2.0, C1S, ALU.mult, ALU.add)
        v.tensor_sub(w3[:], uxy, w1[:])
        v.tensor_scalar(w1[:], w3[:], 2.0 * COV, C2S, ALU.mult, ALU.add)
        v.tensor_scalar(w3[:], w4[:], 1.0, C1S, ALU.mult, ALU.add)
        v.tensor_add(w5[:], uxx, uyy)
        v.tensor_sub(w5[:], w5[:], w4[:])
        v.tensor_scalar(w5[:], w5[:], COV, C2S, ALU.mult, ALU.add)
        v.tensor_mul(w2[:], w2[:], w1[:])
        v.tensor_mul(w3[:], w3[:], w5[:])
        w6 = wA_pool.tile([BL, 81], f32, tag="wa", bufs=6, name=f"w6c{c}")
        sc.activation(w3[:], w3[:], AF.Ln)
        sc.activation(w6[:], w3[:], AF.Exp, scale=-1.0)
        v.tensor_mul(SS[:, c, :], w2[:], w6[:])
        v.tensor_mul(w1[:], SS[:, c, :], TT34[:, c, :])
        v.tensor_add(Z[:, c, :], w1[:], TT34[:, 2 + c, :])

    F22T = pp.tile([BL, 81], f32, name="F22T")
    SSCC = pp.tile([BL, 81], f32, name="SSCC")
    wz = wA_pool.tile([BL, 81], f32, tag="wa", bufs=6, name="wz")
    v.tensor_scalar(wz[:], Z[:, 1, :], BV[0:BL, BV_W01 : BV_W01 + 1],
                    BV[0:BL, BV_BCC : BV_BCC + 1], ALU.mult, ALU.add)
    v.scalar_tensor_tensor(F22T[:], Z[:, 0, :],
                           BV[0:BL, BV_W00 : BV_W00 + 1], wz[:],
                           ALU.mult, ALU.add)
    wz2 = wA_pool.tile([BL, 81], f32, tag="wa", bufs=6, name="wz2")
    v.tensor_scalar(wz2[:], SS[:, 1, :], BV[0:BL, BV_W01 : BV_W01 + 1],
                    BV[0:BL, BV_BCC : BV_BCC + 1], ALU.mult, ALU.add)
    v.scalar_tensor_tensor(SSCC[:], SS[:, 0, :],
                           BV[0:BL, BV_W00 : BV_W00 + 1], wz2[:],
                           ALU.mult, ALU.add)

    stage_f1(1, [3, 4, 5])

    # -- A5: fc1 (pool conv folded on host) --------------------------
    ptr = ps_a.tile([81, BL], f32, tag="pa", name="ptrF22")
    te.transpose(ptr[:], F22T[:], IDF[0:BL, 0:BL])
    F22TT = pp.tile([81, BL], bf16, name="F22TT")
    sc.activation(F22TT[:], ptr[:], AF.Copy)

    H1S = pp.tile([128, 3, BL], bf16, name="H1S")
    nc.gpsimd.memset(H1S[:], 0.0)
    for mo in range(3):
        osz = min(128, 324 - mo * 128)
        pf = ps_a.tile([128, BL], f32, tag="pa", name=f"pf{mo}")
        te.matmul(pf[0:osz, :], WFC1[:, mo * 128 : mo * 128 + osz],
                  F22TT[:], start=True, stop=True)
        sc.activation(H1S[0:osz, mo, :], pf[0:osz, :], AF.Gelu,
                      bias=BV[0:osz, BV_BFC1 + mo : BV_BFC1 + mo + 1])

    stage_f2(0, [0, 1, 2])

    # -- A6: fc2 + leaky + linearize ---------------------------------
    pxw = ps_a.tile([81, BL], f32, tag="pa", name="pxw")
    for k in range(3):
        te.matmul(pxw[:], WFC2[:, k, :], H1S[:, k, :],
                  start=(k == 0), stop=(k == 2))
    XWT = pp.tile([81, BL], f32, name="XWT")
    sc.activation(XWT[:], pxw[:], AF.Lrelu,
                  bias=BV[0:81, BV_BFC2 : BV_BFC2 + 1], alpha=0.01)
    ptw = ps_a.tile([BL, 81], f32, tag="pa", name="ptw")
    te.transpose(ptw[:], XWT[:], IDF[0:81, 0:81])
    XWB = pp.tile([BL, 81], f32, name="XWB")
    sc.activation(XWB[:], ptw[:], AF.Copy)

    # linearize [BL, 81] -> b-major rows staged in DRAM; tiles load slices
    lin_scr = []
    for i, srct in enumerate((F22T, SSCC, XWB)):
        cb = wA_pool.tile([BL, 81], bf16, tag="wcb", name=f"cb{i}")
        v.tensor_copy(cb[:], srct[:, :])
        scr = nc.dram_tensor(f"lin_scr{i}", [BL, 81], bf16, kind="Internal")
        nc.sync.dma_start(out=scr.ap(), in_=cb[:, :])
        lin_scr.append(scr)

    stage_f2(0, [3, 4, 5])

    # conv weights arrive while passes run; split across queues
    wb_re = io["wb"].ap().rearrange("d (kc p) m -> p d kc m", p=128)
    for d in range(9):
        nc.sync.dma_start(out=WB[:, d, :, :], in_=wb_re[:, d, :, :])

    for g in range(N_TILES):
        stage_lrows(g)
        if g >= 1:
            if g + 1 < N_TILES:
                stage_load(g + 1)
            stage_bconv(g - 1, [0])
        stage_fold_red(g, "r2")
        if g >= 1 and g + 1 < N_TILES:
            stage_f1(g + 1, [0])
        stage_fold_red(g, "r3")
        stage_cor1_q(g)
        if g >= 1 and g + 1 < N_TILES:
            stage_f1(g + 1, [1])
        stage_fold_red(g, "r1")
        if g >= 1 and g + 1 < N_TILES:
            stage_f1(g + 1, [2])
        if g >= 1:
            stage_bconv(g - 1, [1])
        stage_cor1_bb(g)
        stage_fold_red(g, "r6")
        if g >= 1 and g + 1 < N_TILES:
            stage_f1(g + 1, [3])
        stage_fold_red(g, "r7")
        if g >= 1 and g + 1 < N_TILES:
            stage_f1(g + 1, [4, 5])
        stage_algebra(g)
        stage_fuse21(g)
        if g >= 1:
            stage_bconv(g - 1, [2])
        if g + 1 < N_TILES:
            stage_f2(g + 1, range(MO))
        stage_cor2(g)
        if g >= 1:
            stage_bconv(g - 1, [3])
        stage_cor2_bcast(g)
        if g >= 1:
            stage_bconv(g - 1, [4])
        stage_xout(g)
        if g >= 1:
            stage_bconv(g - 1, [5])
    stage_bconv(N_TILES - 1, list(range(MO)))

def _split_excess_waits(nc, limit=_SYNC_WAIT_LIMIT):
    """walrus allows only a couple of sem waits per instruction; move any
    excess onto same-engine nops inserted right before the instruction."""
    import bass_rust

    cnt = 0
    for f in nc.m.functions:
        for b in f.blocks:
            insts = b.instructions
            newlist = []
            changed = False
            for inst in insts:
                si = getattr(inst, "sync_info", None)
                waits = list(si.on_wait) if si is not None else []
                if len(waits) > limit:
                    changed = True
                    extra, keep = waits[:-limit], waits[-limit:]
                    for j in range(0, len(extra), limit):
                        nop = mybir.InstNoOp(name=f"waitnop_{cnt}", ins=[],
                                             outs=[])
                        cnt += 1
                        nop.engine = inst.engine
                        nop.sync_info = bass_rust.SyncInfo(
                            on_wait=extra[j : j + limit], on_update=[])
                        nc.register_instruction(nop, overwrite=True)
                        newlist.append(nop)
                    inst.sync_info = bass_rust.SyncInfo(
                        on_wait=keep, on_update=list(si.on_update))
                newlist.append(inst)
            if changed:
                insts[:] = newlist


_PROGRAM_CACHE = {}


def _build_program():
    if "nc" in _PROGRAM_CACHE:
        return _PROGRAM_CACHE["nc"]
    _patch_drain_wait_limit()
    nc = bass.Bass("TRN2", target_bir_lowering=False, debug=False,
                   num_devices=1)
    io = {}
    io["xe"] = nc.dram_tensor("xe", [C, NE], bf16, kind="ExternalInput")
    io["ye"] = nc.dram_tensor("ye", [L, NE], bf16, kind="ExternalInput")
    io["wh1"] = nc.dram_tensor("wh1", [C, C], bf16, kind="ExternalInput")
    io["wf2x"] = nc.dram_tensor("wf2x", [C, C], bf16, kind="ExternalInput")
    io["wf2y"] = nc.dram_tensor("wf2y", [L, C], bf16, kind="ExternalInput")
    io["a3x"] = nc.dram_tensor("a3x", [C, 2], bf16, kind="ExternalInput")
    io["sy4"] = nc.dram_tensor("sy4", [L, 4], bf16, kind="ExternalInput")
    io["wb"] = nc.dram_tensor("wb", [9, C, O], bf16, kind="ExternalInput")
    io["wfc1"] = nc.dram_tensor("wfc1", [81, 324], bf16, kind="ExternalInput")
    io["wfc2"] = nc.dram_tensor("wfc2", [384, 81], bf16, kind="ExternalInput")
    io["shw"] = nc.dram_tensor("shw", [81, 81], f32, kind="ExternalInput")
    io["sb"] = nc.dram_tensor("sb", [BE, BL], f32, kind="ExternalInput")
    io["bv"] = nc.dram_tensor("bv", [128, BV_NCOLS], f32, kind="ExternalInput")
    io["out"] = nc.dram_tensor("out", [O, NV], f32, kind="ExternalOutput")

    from contextlib import ExitStack

    with tile.TileContext(nc) as tc, ExitStack() as ctx:
        _emit(ctx, nc, tc, io)
    _split_excess_waits(nc)
    _PROGRAM_CACHE["nc"] = nc
    return nc


def _reflect_filter_1d(n, win):
    """uniform_filter1d with reflect ('symmetric') padding as an n x n map."""
    r = win // 2
    s = np.zeros((n, n), np.float64)
    for o in range(n):
        for k in range(o - r, o + r + 1):
            i = k
            if i < 0:
                i = -i - 1
            if i > n - 1:
                i = 2 * n - 1 - i
            s[o, i] += 1.0 / win
    return s


def host_prepare(inputs):
    f64 = np.float64
    x = np.asarray(inputs["x"], np.float32)
    y = np.asarray(inputs["y"], np.float32)
    W11 = np.asarray(inputs["w_conv1_1"], f64)
    wf2x = (W11[:, :C2] @ np.asarray(inputs["w_convh2"], f64)).astype(np.float32)
    wf2y = (W11[:, C2:] @ np.asarray(inputs["w_convl1"], f64)).astype(np.float32)
    b_f2 = (W11[:, :C2] @ np.asarray(inputs["b_convh2"], f64)
            + W11[:, C2:] @ np.asarray(inputs["b_convl1"], f64)
            + np.asarray(inputs["b_conv1_1"], f64)).astype(np.float32)
    w12 = np.asarray(inputs["w_conv1_2"], f64)
    a3x = (w12[:, 0:1] @ np.asarray(inputs["w_convh3"], f64)).astype(np.float32)
    a3y = (w12[:, 1:2] @ np.asarray(inputs["w_convl2"], f64)).astype(np.float32)
    b3 = (w12 @ np.concatenate([np.asarray(inputs["b_convh3"], f64),
                                np.asarray(inputs["b_convl2"], f64)])
          + np.asarray(inputs["b_conv1_2"], f64)).astype(np.float32)
    bias4 = np.concatenate([b3, np.asarray(inputs["b_convl3"], np.float32)])

    sy4 = np.concatenate(
        [a3y.T, np.asarray(inputs["w_convl3"], np.float32).T], axis=1)

    s1 = _reflect_filter_1d(HH, WIN)
    shw = np.kron(s1, s1).T.astype(np.float32)  # lhsT [in_px, out_px]
    sb_m = np.zeros((BE, BL), np.float32)
    for o in range(BL):
        sb_m[o : o + WIN, o] = 1.0 / WIN

    w_pool = np.asarray(inputs["w_pool"], f64)  # (2, 1, 3, 3)
    mconv = np.zeros((2, 81, 81), f64)          # [c, out_px, in_px]
    for c in range(2):
        for oh in range(HH):
            for ow in range(WW):
                for dh in range(3):
                    for dw in range(3):
                        ih, iw = oh + dh - 1, ow + dw - 1
                        if 0 <= ih < HH and 0 <= iw < WW:
                            mconv[c, oh * WW + ow, ih * WW + iw] = \
                                w_pool[c, 0, dh, dw]

    bfd = ml_dtypes.bfloat16
    W1 = np.asarray(inputs["w_fc1"], f64)
    bp = np.asarray(inputs["b_pool"], f64)
    wf = (W1[:, 0:81] + W1[:, 243:324]
          + W1[:, 81:162] @ mconv[0] + W1[:, 162:243] @ mconv[1])
    wfc1 = np.ascontiguousarray(wf.T).astype(bfd)       # lhsT [81, 324]
    bfc1 = (np.asarray(inputs["b_fc1"], f64)
            + bp[0] * W1[:, 81:162].sum(axis=1)
            + bp[1] * W1[:, 162:243].sum(axis=1)).astype(np.float32)
    wfc2 = np.zeros((384, 81), bfd)
    wfc2[:324] = np.asarray(inputs["w_fc2"], np.float32).T.astype(bfd)

    bn_scale = (np.asarray(inputs["bn_gamma"], f64)
                / np.sqrt(np.asarray(inputs["bn_var"], f64) + 1e-5))
    bn_bias = (np.asarray(inputs["bn_beta"], f64)
               - np.asarray(inputs["bn_mean"], f64) * bn_scale)

    bv = np.zeros((128, BV_NCOLS), np.float32)
    b_h1 = np.asarray(inputs["b_convh1"], np.float32)
    for m in range(MO):
        bv[:, BV_BH1 + m] = b_h1[m * 128 : (m + 1) * 128]
        bv[:, BV_BF2 + m] = b_f2[m * 128 : (m + 1) * 128]
        bv[:, BV_BNS + m] = bn_scale[m * 128 : (m + 1) * 128]
        bv[:, BV_BNB + m] = bn_bias[m * 128 : (m + 1) * 128]
    bv[0:2, BV_B4] = bias4[0:2]
    bv[0:2, BV_B4Y] = bias4[2:4]
    for mo in range(3):
        osz = min(128, 324 - mo * 128)
        bv[0:osz, BV_BFC1 + mo] = bfc1[mo * 128 : mo * 128 + osz]
    bv[0:81, BV_BFC2] = np.asarray(inputs["b_fc2"], np.float32)
    bv[:, BV_W00] = np.float32(inputs["w_cc1"][0, 0])
    bv[:, BV_W01] = np.float32(inputs["w_cc1"][0, 1])
    bv[:, BV_BCC] = np.float32(inputs["b_cc1"][0])
    bv[:, BV_BP0] = np.float32(inputs["b_pool"][0])
    bv[:, BV_BP1] = np.float32(inputs["b_pool"][1])

    bf = ml_dtypes.bfloat16
    common = {
        "wh1": np.asarray(inputs["w_convh1"], np.float32).T.astype(bf),
        "wf2x": wf2x.T.astype(bf),
        "wf2y": wf2y.T.astype(bf),
        "a3x": a3x.T.astype(bf),
        "sy4": sy4.astype(bf),
        "wb": np.asarray(inputs["w_bconv"], np.float32)
              .transpose(2, 3, 1, 0).reshape(9, C, O).astype(bf),
        "wfc1": wfc1, "wfc2": wfc2,
        "shw": shw, "sb": sb_m, "bv": bv,
    }
    common = {k: np.ascontiguousarray(v) for k, v in common.items()}

    xp = np.pad(x, ((HALO, HALO), (0, 0), (0, 0), (0, 0)), mode="symmetric")
    yp = np.pad(y, ((HALO, HALO), (0, 0), (0, 0), (0, 0)), mode="symmetric")
    in_maps = []
    for m in range(M_CORES):
        xe = np.ascontiguousarray(
            xp[m * BL : m * BL + BE].transpose(1, 0, 2, 3).reshape(C, NE)
        ).astype(bf)
        ye = np.ascontiguousarray(
            yp[m * BL : m * BL + BE].transpose(1, 0, 2, 3).reshape(L, NE)
        ).astype(bf)
        in_maps.append({"xe": xe, "ye": ye, **common})
    return in_maps


def kernel(**inputs):
    nc = _build_program()
    in_maps = host_prepare(inputs)
    trace = os.environ.get("KERNEL_TRACE", "0") == "1"
    kw = {}
    if trace:
        kw = dict(trace=True, trace_cores=[0])
    res = run_bass_kernel_spmd(nc, in_maps, core_ids=list(range(M_CORES)), **kw)
    if trace:
        kernel.last_results = res
        if res.exec_time_ns is not None:
            print(f"HW exec time: {res.exec_time_ns} ns")
    out = np.empty((B, O, HH, WW), np.float32)
    for m in range(M_CORES):
        o = res.results[m]["out"]
        out[m * BL : (m + 1) * BL] = (
            o.reshape(O, BL, HH, WW).transpose(1, 0, 2, 3))
    return out

